# revision 22
# baseline (speedup 1.0000x reference)
"""Trainium2 Bass kernel for chunked-local GQA attention (nn_Attention_12266426597578).

Full-input contract: kernel(**inputs) takes the unsharded numpy inputs
(x [1,4096,4096], wq [4096,4096], wk [1024,4096], wv [1024,4096],
wo [4096,4096]) and returns the full output [1,4096,4096].

Sharding (8 cores): 2 chunk-groups (2 attention chunks of 1024 tokens each)
x 4 head-groups (8 q-heads / 2 kv-heads each). Each core computes a partial
y^T [D, 2048] for its token range; the host sums the 4 head-group partials
per token range and concatenates.

Device-side layout: transposed (feature dim on SBUF partitions, tokens on
the free axis) so QKV projections, RoPE (128x128 pair-rotation matmul),
RMSNorm reductions (ones-vector matmuls), scores^T, softmax denominators
and PV all map onto the PE array without on-device transposition of x or
the weights (the host pre-transposes).

This version software-pipelines the per-chunk phases across the two chunks
so the PE array never waits on the ACT-bound attention phase:

    P0(+post0) -> A0 (x) P1raw -> post1 (x) W0[head] -> A1 (x) W0[rest] -> W1

where (x) denotes instruction-level interleaving driven by generators.
Other changes vs the phase-serial baseline: score pairs land in a single
2-bank PSUM tile so each j-block needs ONE exp ACTIVATE; row-replication
(q-norm rows, softmax reciprocal rows) is done with partition-broadcast
DMAs instead of PE matmuls; wo outputs are DMA'd to DRAM directly from
PSUM; the QKV projection runs in 2 d-passes (NQPASS=2) with bf16 partial
sums to halve the resident x working set.
"""

import sys

sys.path.insert(0, "/opt/trn_rl_repo")

import numpy as np
import ml_dtypes
from contextlib import ExitStack

import concourse.bass as bass  # noqa: F401
import concourse.tile as tile
from concourse import bacc, mybir
from concourse.bass_utils import run_bass_kernel_spmd
import concourse.bacc as _bacc_mod
from concourse.hw_specs import get_activation_tables as _gat_orig


def _gat_combined(arch):
    # Restrict the ACT table-load pass to the one set that holds square,
    # ln, exp and copy together, so the kernel never reloads ACT tables.
    t = _gat_orig(arch)
    return {
        k: (v if k == "natural_log_exp_and_others" else set())
        for k, v in t.items()
    }


_bacc_mod.get_activation_tables = _gat_combined

# Problem constants (hardcoded per contract)
S, D = 4096, 4096
H, KVH, HD = 32, 8, 128
CHUNK = 1024
EPS = 1e-5
THETA = 500000.0
ISQ_HD = 1.0 / np.sqrt(np.float32(HD))

# Sharding
NCORES = 8
CG = 2  # chunk groups (token split)
HG = 4  # head groups
TOK = S // CG  # 2048 tokens per core
NCH = TOK // CHUNK  # 2 chunks per core
QH = H // HG  # 8 q heads per core
KH = KVH // HG  # 2 kv heads per core
QO = QH * HD  # 1024
KO = KH * HD  # 256
QKVO = QO + 2 * KO  # 1536
NOT = QKVO // 128  # 12 o-tiles: 0..1 k, 2..3 v, 4..11 q
NB = CHUNK // 128  # 8 blocks of 128 tokens per chunk

f32 = mybir.dt.float32
f32r = mybir.dt.float32r
bf16 = mybir.dt.bfloat16

NPASS = 2  # d-accumulation passes (bf16 SBUF partial sums)
DPP = D // 128 // NPASS  # d-tiles per pass (16)
_CACHE = {}

SDT = bf16
SNP = ml_dtypes.bfloat16

_DONE = object()


def _drive(gen, n):
    """Advance a generator by n yield-steps; returns False when exhausted."""
    for _ in range(n):
        if next(gen, _DONE) is _DONE:
            return False
    return True


def _drain(gen):
    for _ in gen:
        pass


class Ctx:
    """Shared emission context for one core program."""

    def __init__(self, nc, sb, ps, consts, dram):
        self.nc = nc
        self.sb = sb
        self.ps = ps
        self.c = consts
        self.xT, self.wqkvT, self.woT, self.yT = dram
        # per-chunk state
        self.qk = {}       # (ci, ot) -> raw qkv tile (bf16)
        self.fin = {}      # (ci, ot) -> rope+norm'd tile (bf16) for k/q
        self.vtok = {}     # (ci, kvh) -> v transposed to token-major
        self.rkln = {}     # (ci, kvh) -> ln(mean sq + eps) per key block
        self.at = {}       # (ci, h) -> attention output tile


# ---------------------------------------------------------------------------
# Phase 1: raw QKV projection (no post-processing)
# ---------------------------------------------------------------------------

def gen_proj(g, ci, inline_post=False):
    """Raw projection matmuls for chunk ci. Yields after every 4-dk group
    (8 matmuls ~= 4096 cols ~= 1.7us of PE). If inline_post, runs post(ot)
    for each o-tile as its accumulation completes (pass 1)."""
    nc, sb, ps = g.nc, g.sb, g.ps
    s0 = ci * CHUNK
    xts = [None] * (NPASS * DPP)
    post = gen_post(g, ci) if inline_post else None

    def xtile(dk):
        if xts[dk] is None:
            xt = sb.tile([128, CHUNK], SDT, name=f"xt_{ci}_{dk}", tag="xt",
                         bufs=DPP + 3)
            # cold start (chunk 0, first pass): the scalar queue is idle, so
            # split the initial x burst across two queues
            eng = (nc.scalar if (ci == 0 and dk < DPP and dk % 2 == 1)
                   else nc.gpsimd)
            eng.dma_start(
                xt[:], g.xT[dk * 128 : (dk + 1) * 128, s0 : s0 + CHUNK]
            )
            xts[dk] = xt
        return xts[dk]

    for ot in range(NOT):
        g.qk[(ci, ot)] = sb.tile([128, CHUNK], bf16, name=f"qkv_{ci}_{ot}",
                                 tag="qk", bufs=NOT + 2)

    for p in range(NPASS):
        for ot in range(NOT):
            wt = sb.tile([128, DPP * 128], SDT, name=f"w_{ci}_{p}_{ot}",
                         tag="w", bufs=4)
            weng = nc.sync if ot % 2 == 0 else nc.gpsimd
            weng.dma_start(wt[:], g.wqkvT[ot, :, p * DPP : (p + 1) * DPP, :])
            if ot == 0:
                # issue the whole pass's x DMAs before any matmul so the
                # PE never waits mid-stream
                for dkk in range(DPP):
                    xtile(p * DPP + dkk)
                yield
            # sh-outer: finish one half-accumulator fully before starting the
            # next, so its PSUM->SBUF copy has a whole half-og of slack before
            # the ring slot is needed again.
            for sh in range(2):
                acc = ps.tile([128, 512], f32, name=f"pj_{ci}_{p}_{ot}_{sh}",
                              tag="pacc", bufs=3)
                for gg in range(DPP // 4):
                    for dkk in range(gg * 4, gg * 4 + 4):
                        xtl = xtile(p * DPP + dkk)
                        nc.tensor.matmul(
                            acc[:],
                            lhsT=wt[:, dkk * 128 : (dkk + 1) * 128],
                            rhs=xtl[:, sh * 512 : (sh + 1) * 512],
                            start=(dkk == 0),
                            stop=(dkk == DPP - 1),
                        )
                    yield
                dst = g.qk[(ci, ot)][:, sh * 512 : (sh + 1) * 512]
                if p == 0:
                    nc.vector.tensor_copy(dst, acc[:])
                else:
                    nc.vector.tensor_tensor(
                        dst, dst, acc[:], mybir.AluOpType.add
                    )
            if inline_post and p == NPASS - 1 and ot >= 1:
                # lag post by one o-tile so its ACT chains never stall the PE
                _drive(post, 1)
    if inline_post:
        _drain(post)


# ---------------------------------------------------------------------------
# Post-processing: RoPE + RMSNorm (q/k), transpose (v)
# ---------------------------------------------------------------------------

def gen_post(g, ci):
    """Per o-tile RoPE/RMSNorm/transpose. Yields once per o-tile."""
    nc, sb, ps = g.nc, g.sb, g.ps
    c = g.c
    s0 = ci * CHUNK
    pending = []

    for ot in range(NOT):
        flush = pending[:]
        pending.clear()
        raw = g.qk[(ci, ot)]
        if 2 <= ot < 4:  # v: PE transpose to token-major, bf16
            kvh = ot - 2
            vt = sb.tile([128, CHUNK], bf16, name=f"v_{ci}_{kvh}", tag="v",
                         bufs=5)
            for b in range(NB):
                tp = ps.tile([128, 128], bf16, name=f"vtp_{ci}_{kvh}_{b}",
                             tag="pv", bufs=2)
                nc.tensor.matmul(
                    tp[:],
                    lhsT=raw[:, b * 128 : (b + 1) * 128],
                    rhs=c["identb"][:],
                    is_transpose=True,
                )
                nc.vector.tensor_copy(vt[:, b * 128 : (b + 1) * 128], tp[:])
            g.vtok[(ci, kvh)] = vt
            for f in flush:
                f()
            yield
            continue

        is_q = ot >= 4
        # sumsq over head-dim partitions (RoPE-invariant -> use raw tile)
        sqh = []
        for sh in range(2):
            t = sb.tile([128, 512], bf16, name=f"sq_{ci}_{ot}_{sh}", tag="sq",
                        bufs=2)
            nc.scalar.activation(t[:], raw[:, sh * 512 : (sh + 1) * 512],
                                 mybir.ActivationFunctionType.Square)
            sqh.append(t)
        smt = ps.tile([128, 512], f32, name=f"ssq_{ci}_{ot}", tag="sm", bufs=1)
        if is_q:
            h = ot - 4
            # rows 0 / 32 of one PSUM bank hold the two 512-halves
            nc.tensor.matmul(smt[0:1, :], lhsT=c["onesb"][:], rhs=sqh[0][:])
            nc.tensor.matmul(smt[32:33, :], lhsT=c["onesb"][:], rhs=sqh[1][:])
            lnr = sb.tile([1, CHUNK], f32, name=f"lnq_{ci}_{ot}", tag="rowq",
                          bufs=2)
            nc.scalar.activation(
                lnr[:, 0:512], smt[0:1, :], mybir.ActivationFunctionType.Ln,
                bias=c["beps"][0:1, :], scale=float(1.0 / HD)
            )
            nc.scalar.activation(
                lnr[:, 512:1024], smt[32:33, :],
                mybir.ActivationFunctionType.Ln,
                bias=c["beps"][0:1, :], scale=float(1.0 / HD)
            )
            rows = sb.tile([1, CHUNK], bf16, name=f"rqr_{ci}_{ot}", tag="rowq",
                           bufs=2)
            nc.scalar.activation(
                rows[:], lnr[:], mybir.ActivationFunctionType.Exp, scale=-0.5
            )
            # replicate rows to all partitions via DMA broadcast
            bc = sb.tile([128, CHUNK], bf16, name=f"bcq_{ci}_{ot}", tag="bc",
                         bufs=3)
            nc.gpsimd.partition_broadcast(bc[:], rows[:])
        else:
            kvh = ot
            for b in range(NB):
                nc.tensor.matmul(
                    smt[:, b : b + 1],
                    lhsT=sqh[b // 4][:, (b % 4) * 128 : (b % 4 + 1) * 128],
                    rhs=c["onesb"][:],
                )
            # ln(mean+eps); rks = exp(-0.5*ln + ln(1/sqrt(HD))) emitted at the
            # attention boundary (same EXP table as the softmax).
            lnk = sb.tile([128, NB], f32, name=f"lnk_{ci}_{kvh}", tag="rk",
                          bufs=10)
            nc.scalar.activation(
                lnk[:], smt[:, 0:NB], mybir.ActivationFunctionType.Ln,
                bias=c["beps"][:, :], scale=float(1.0 / HD)
            )
            g.rkln[(ci, kvh)] = lnk

        # RoPE (fp32 math): fin <- (raw*cos + (R @ raw)*sin) [*1/rms for q]
        fin = sb.tile([128, CHUNK], bf16, name=f"fin_{ci}_{ot}", tag="qb",
                      bufs=14)
        for sh in range(2):
            rot = ps.tile([128, 512], f32, name=f"rot_{ci}_{ot}_{sh}",
                          tag="pv", bufs=2)
            nc.tensor.matmul(
                rot[:], lhsT=c["rmat"][:], rhs=raw[:, sh * 512 : (sh + 1) * 512]
            )
            nc.vector.tensor_tensor(
                rot[:], rot[:],
                g.c["sintab"][:, s0 + sh * 512 : s0 + (sh + 1) * 512],
                mybir.AluOpType.mult,
            )
            nc.vector.tensor_tensor(
                raw[:, sh * 512 : (sh + 1) * 512],
                raw[:, sh * 512 : (sh + 1) * 512],
                g.c["costab"][:, s0 + sh * 512 : s0 + (sh + 1) * 512],
                mybir.AluOpType.mult,
            )
            nc.vector.tensor_tensor(
                fin[:, sh * 512 : (sh + 1) * 512],
                raw[:, sh * 512 : (sh + 1) * 512],
                rot[:],
                mybir.AluOpType.add,
            )
        if is_q:
            def fin_q(fin=fin, bc=bc):
                nc.vector.tensor_tensor(
                    fin[:], fin[:], bc[:], mybir.AluOpType.mult
                )
            pending.append(fin_q)
        g.fin[(ci, ot)] = fin
        for f in flush:
            f()
        yield
    for f in pending:
        f()


# ---------------------------------------------------------------------------
# Phase 2: attention (block-causal within chunk)
# ---------------------------------------------------------------------------

def gen_attn(g, ci):
    """Yields once per (head, j-block) plus once per head tail."""
    nc, sb, ps = g.nc, g.sb, g.ps
    c = g.c
    hpending = []
    for kvh in range(KH):
        kf = g.fin[(ci, kvh)]
        vt = g.vtok[(ci, kvh)]
        # rks = exp(-0.5*ln(mean+eps) + ln(1/sqrt(HD))) = 1/(rms*sqrt(HD))
        rks = sb.tile([128, NB], f32, name=f"rks_{ci}_{kvh}", tag="rk",
                      bufs=10)
        nc.scalar.activation(
            rks[:], g.rkln[(ci, kvh)][:], mybir.ActivationFunctionType.Exp,
            bias=c["blniq"][:, :], scale=-0.5,
        )
        for h4 in range(4):
            h = kvh * 4 + h4
            qf = g.fin[(ci, 4 + h)]
            pvA = ps.tile([128, 512], f32, name=f"pvA_{ci}_{h}", tag="pv",
                          bufs=2)
            pvB = ps.tile([128, 512], f32, name=f"pvB_{ci}_{h}", tag="pv",
                          bufs=2)
            # two sum rows packed in one PSUM bank; head parity picks rows
            # (0,32) vs (64,96) so consecutive heads don't serialize.
            smt = ps.tile([128, 512], f32, name=f"sm_{ci}_{h}", tag="sm",
                          bufs=1) if h % 2 == 0 else None
            if smt is not None:
                g._smt = smt
            smt = g._smt
            ra = (h % 2) * 64
            rb = ra + 32
            at = sb.tile([128, CHUNK], SDT, name=f"attn_{ci}_{h}", tag="attn",
                         bufs=18)
            for j in range(NB):
                w = CHUNK - j * 128
                c0 = j * 128
                lenA = 512 - c0 if j < 4 else 0
                b0 = max(512, c0)
                sc = ps.tile([128, CHUNK], f32, name=f"sc_{ci}_{h}_{j}",
                             tag="sc", bufs=1)
                pt = sb.tile([128, CHUNK], bf16, name=f"pt_{ci}_{h}_{j}",
                             tag="pt", bufs=3)
                if lenA > 0:
                    nc.tensor.matmul(
                        sc[:, c0:512],
                        lhsT=kf[:, c0 : c0 + 128],
                        rhs=qf[:, c0:512],
                    )
                    nc.tensor.matmul(
                        sc[:, 512:CHUNK],
                        lhsT=kf[:, c0 : c0 + 128],
                        rhs=qf[:, 512:CHUNK],
                    )
                    # one exp over the contiguous [c0:1024) score range
                    nc.scalar.activation(
                        pt[:, 0:w], sc[:, c0:CHUNK],
                        mybir.ActivationFunctionType.Exp,
                        scale=rks[:, j : j + 1],
                    )
                else:
                    nc.tensor.matmul(
                        sc[:, 0:w],
                        lhsT=kf[:, c0 : c0 + 128],
                        rhs=qf[:, b0:CHUNK],
                    )
                    nc.scalar.activation(
                        pt[:, 0:w], sc[:, 0:w],
                        mybir.ActivationFunctionType.Exp,
                        scale=rks[:, j : j + 1],
                    )
                yield  # interleave point: ACT computes exp meanwhile
                nc.vector.tensor_tensor(
                    pt[:, 0:128], pt[:, 0:128], c["mask"][:],
                    mybir.AluOpType.mult
                )
                if lenA > 0:
                    nc.tensor.matmul(
                        smt[ra : ra + 1, c0:512], lhsT=c["onesb"][:],
                        rhs=pt[:, 0:lenA],
                        start=(j == 0), stop=(j == 3),
                        tile_position=(0, ra),
                    )
                nc.tensor.matmul(
                    smt[rb : rb + 1, b0 - 512 : 512], lhsT=c["onesb"][:],
                    rhs=pt[:, b0 - c0 : w],
                    start=(j == 0), stop=(j == NB - 1),
                    tile_position=(0, rb),
                )
                if lenA > 0:
                    nc.tensor.matmul(
                        pvA[:, c0:512],
                        lhsT=vt[:, c0 : c0 + 128],
                        rhs=pt[:, 0:lenA],
                        start=(j == 0), stop=(j == 3),
                    )
                    if j == 3:
                        # evacuate the finished A half right away to free the
                        # PSUM bank for the next head
                        nc.vector.tensor_copy(at[:, 0:512], pvA[:])
                nc.tensor.matmul(
                    pvB[:, b0 - 512 : 512],
                    lhsT=vt[:, c0 : c0 + 128],
                    rhs=pt[:, b0 - c0 : w],
                    start=(j == 0), stop=(j == NB - 1),
                )
                if j == NB - 1:
                    nc.vector.tensor_copy(at[:, 512:1024], pvB[:])
            # denominators: 1/sum = exp(-ln(sum)); broadcast by DMA; the
            # normalize multiply is deferred one head so PE never stalls.
            lns = sb.tile([1, CHUNK], f32, name=f"lns_{ci}_{h}", tag="rowq",
                          bufs=2)
            nc.scalar.activation(lns[:, 0:512], smt[ra : ra + 1, :],
                                 mybir.ActivationFunctionType.Ln)
            nc.scalar.activation(lns[:, 512:1024], smt[rb : rb + 1, :],
                                 mybir.ActivationFunctionType.Ln)
            srows = sb.tile([1, CHUNK], bf16, name=f"srows_{ci}_{h}",
                            tag="srow", bufs=2)
            nc.scalar.activation(
                srows[:], lns[:], mybir.ActivationFunctionType.Exp, scale=-1.0
            )
            bcs = sb.tile([128, CHUNK], bf16, name=f"bcs_{ci}_{h}", tag="bc",
                          bufs=3)
            nc.gpsimd.partition_broadcast(bcs[:], srows[:])
            flush = hpending[:]
            hpending.clear()
            for f in flush:
                f()

            def fin_head(at=at, bcs=bcs):
                nc.vector.tensor_tensor(
                    at[:], at[:], bcs[:], mybir.AluOpType.mult
                )
            hpending.append(fin_head)
            g.at[(ci, h)] = at
            yield
    for f in hpending:
        f()


# ---------------------------------------------------------------------------
# Phase 3: output projection (y DMA'd straight from PSUM)
# ---------------------------------------------------------------------------

def gen_wo(g, ci):
    """Yields 4x per dd tile (after every 4 accumulation matmuls)."""
    nc, sb, ps = g.nc, g.sb, g.ps
    s0 = ci * CHUNK
    wobs = {}

    def wob_load(dd):
        if dd not in wobs and dd < 32:
            t = sb.tile([128, QO], SDT, name=f"wo_{ci}_{dd}", tag="wo", bufs=4)
            nc.gpsimd.dma_start(t[:], g.woT[dd])
            wobs[dd] = t
        return wobs.get(dd)

    for dd in range(32):
        wob = wob_load(dd)
        wob_load(dd + 1)  # double-buffer the next weight block
        ysb = sb.tile([128, CHUNK], f32, name=f"ysb_{ci}_{dd}", tag="y", bufs=2)
        for sh in range(2):
            yps = ps.tile([128, 512], f32, name=f"y_{ci}_{dd}_{sh}",
                          tag="pacc", bufs=3)
            for h in range(QH):
                nc.tensor.matmul(
                    yps[:],
                    lhsT=wob[:, h * 128 : (h + 1) * 128],
                    rhs=g.at[(ci, h)][:, sh * 512 : (sh + 1) * 512],
                    start=(h == 0), stop=(h == QH - 1),
                )
                if h % 4 == 3:
                    yield
            nc.vector.tensor_copy(
                ysb[:, sh * 512 : (sh + 1) * 512], yps[:]
            )
            # split the output stream across two DMA queues so neither
            # saturates during the wo phases
            yeng = nc.sync if sh == 0 else nc.gpsimd
            yeng.dma_start(
                g.yT[dd * 128 : (dd + 1) * 128,
                     s0 + sh * 512 : s0 + (sh + 1) * 512],
                ysb[:, sh * 512 : (sh + 1) * 512],
            )


def _interleave(main, aux, n_main, n_aux):
    """Drive main to exhaustion, advancing aux proportionally (n_aux aux
    steps spread over n_main main steps)."""
    done_aux = 0
    i = 0
    while True:
        if next(main, _DONE) is _DONE:
            break
        i += 1
        tgt = min(n_aux, i * n_aux // max(1, n_main))
        if tgt > done_aux:
            if _drive(aux, tgt - done_aux):
                done_aux = tgt
            else:
                done_aux = n_aux


def _build_program():
    nc = bacc.Bacc(
        "TRN2",
        target_bir_lowering=False,
        debug=False,
        enable_asserts=False,
        num_devices=NCORES,
    )
    xT = nc.dram_tensor("xT", [D, TOK], SDT, kind="ExternalInput").ap()
    # [ot, p, dk, c]: per (ot, pass) slice is one contiguous-per-partition DMA
    wqkvT = nc.dram_tensor(
        "wqkvT", [NOT, 128, D // 128, 128], SDT, kind="ExternalInput"
    ).ap()
    # per-dd-block tiled layout: woT[dd, p, h*128+c] = wo[dd*128+c, hg*QO + h*128+p]
    woT = nc.dram_tensor("woT", [D // 128, 128, QO], SDT, kind="ExternalInput").ap()
    costab = nc.dram_tensor("costab", [HD, TOK], bf16, kind="ExternalInput").ap()
    sintab = nc.dram_tensor("sintab", [HD, TOK], bf16, kind="ExternalInput").ap()
    rmat = nc.dram_tensor("rmat", [128, 128], bf16, kind="ExternalInput").ap()
    identb = nc.dram_tensor("identb", [128, 128], bf16, kind="ExternalInput").ap()
    mask = nc.dram_tensor("mask", [128, 128], bf16, kind="ExternalInput").ap()
    onesb = nc.dram_tensor("onesb", [128, 1], bf16, kind="ExternalInput").ap()
    beps = nc.dram_tensor("beps", [128, 1], f32, kind="ExternalInput").ap()
    blniq = nc.dram_tensor("blniq", [128, 1], f32, kind="ExternalInput").ap()
    yT = nc.dram_tensor("yT", [D, TOK], f32, kind="ExternalOutput").ap()

    with tile.TileContext(nc) as tc, ExitStack() as ctx:
        ctx.enter_context(nc.allow_low_precision(reason="bf16 attention operands"))
        sb = ctx.enter_context(tc.tile_pool(name="sb", bufs=1))
        ps = ctx.enter_context(tc.tile_pool(name="ps", bufs=1, space="PSUM"))
        cp = ctx.enter_context(tc.tile_pool(name="cp", bufs=1))

        consts = {}
        for nm, ap_, shape, dt_ in (
            ("costab", costab, [HD, TOK], bf16),
            ("sintab", sintab, [HD, TOK], bf16),
            ("rmat", rmat, [128, 128], bf16),
            ("identb", identb, [128, 128], bf16),
            ("mask", mask, [128, 128], bf16),
            ("onesb", onesb, [128, 1], bf16),
            ("beps", beps, [128, 1], f32),
            ("blniq", blniq, [128, 1], f32),
        ):
            t = cp.tile(shape, dt_, name=f"c_{nm}")
            nc.scalar.dma_start(t[:], ap_[:])
            consts[nm] = t

        g = Ctx(nc, sb, ps, consts, (xT, wqkvT, woT, yT))

        # P0 with inline post
        _drain(gen_proj(g, 0, inline_post=True))
        # A0 (x) P1raw: ~72 attn steps over 194 proj yield units
        gp1 = gen_proj(g, 1, inline_post=False)
        _interleave(gen_attn(g, 0), gp1, n_main=72, n_aux=194)
        _drain(gp1)
        # post1 (x) first dds of W0: 12 post steps over 44 wo units (11 dd)
        gw0 = gen_wo(g, 0)
        _drive(gw0, 4)  # first dd fully in flight before post1's chains
        _interleave(gen_post(g, 1), gw0, n_main=12, n_aux=52)
        # A1 (x) rest of W0 (~72 wo units over 72 attn steps)
        _interleave(gen_attn(g, 1), gw0, n_main=72, n_aux=72)
        _drain(gw0)
        # W1
        _drain(gen_wo(g, 1))

    nc.compile()
    return nc


def _host_inputs(x, wq, wk, wv, wo):
    xf = np.ascontiguousarray(x.reshape(S, D).T.astype(SNP))  # [D, S]
    half = HD // 2
    inv_freq = (1.0 / (THETA ** (np.arange(0, half, dtype=np.float32) / half))).astype(
        np.float32
    )
    ang = np.arange(S, dtype=np.float32)[:, None] * inv_freq[None, :]
    cos = np.cos(ang).astype(np.float32)
    sin = np.sin(ang).astype(np.float32)
    costab = np.empty((HD, S), np.float32)
    sintab = np.empty((HD, S), np.float32)
    costab[0::2, :] = cos.T
    costab[1::2, :] = cos.T
    sintab[0::2, :] = sin.T
    sintab[1::2, :] = sin.T
    costab = costab.astype(ml_dtypes.bfloat16)
    sintab = sintab.astype(ml_dtypes.bfloat16)

    rmat = np.zeros((128, 128), ml_dtypes.bfloat16)
    for i in range(64):
        rmat[2 * i + 1, 2 * i] = -1.0
        rmat[2 * i, 2 * i + 1] = 1.0
    identb = np.eye(128, dtype=ml_dtypes.bfloat16)
    mask = np.triu(np.ones((128, 128), np.float32)).astype(ml_dtypes.bfloat16)
    onesb = np.ones((128, 1), ml_dtypes.bfloat16)
    beps = np.full((128, 1), EPS, np.float32)
    blniq = np.full((128, 1), np.log(ISQ_HD), np.float32)

    xT_cg = [np.ascontiguousarray(xf[:, cg * TOK : (cg + 1) * TOK]) for cg in range(CG)]
    cos_cg = [np.ascontiguousarray(costab[:, cg * TOK : (cg + 1) * TOK]) for cg in range(CG)]
    sin_cg = [np.ascontiguousarray(sintab[:, cg * TOK : (cg + 1) * TOK]) for cg in range(CG)]
    wqkvT_hg = []
    woT_hg = []
    for hg in range(HG):
        wq_c = wq[hg * QO : (hg + 1) * QO]
        wk_c = wk[hg * KO : (hg + 1) * KO]
        wv_c = wv[hg * KO : (hg + 1) * KO]
        # o-tile order on device: [k0 k1 v0 v1 q0..q7]
        wcat = np.concatenate([wk_c, wv_c, wq_c], 0)  # [QKVO, D]
        # wqkvT[ot, p, dk, c] = wcat[ot*128 + c?, ...]: lhsT tile for (ot, dk)
        # is w[dk*128+p(part), ot*128+c(free)] = wcat[ot*128+c, dk*128+p].
        wr = wcat.T.reshape(D // 128, 128, NOT, 128)  # [dk, p, ot, c]
        wr = wr.transpose(2, 1, 0, 3)  # [ot, p, dk, c]
        wqkvT_hg.append(np.ascontiguousarray(wr.astype(SNP)))
        wo_c = wo[:, hg * QO : (hg + 1) * QO]  # [D, QO]
        woH = wo_c.reshape(D // 128, 128, QH, 128).transpose(0, 3, 2, 1)  # [dd, p, hb, c]
        woT_hg.append(np.ascontiguousarray(woH.reshape(D // 128, 128, QO).astype(SNP)))

    in_maps = []
    for c in range(NCORES):
        cg, hg = c // HG, c % HG
        in_maps.append(
            {
                "xT": xT_cg[cg],
                "wqkvT": wqkvT_hg[hg],
                "woT": woT_hg[hg],
                "costab": cos_cg[cg],
                "sintab": sin_cg[cg],
                "rmat": rmat,
                "identb": identb,
                "mask": mask,
                "onesb": onesb,
                "beps": beps,
                "blniq": blniq,
            }
        )
    return in_maps


def _assemble(results):
    y = np.empty((S, D), np.float32)
    for cg in range(CG):
        acc = results[cg * HG]["yT"].astype(np.float32)
        for hg in range(1, HG):
            acc = acc + results[cg * HG + hg]["yT"]
        y[cg * TOK : (cg + 1) * TOK, :] = acc.T
    return y.reshape(1, S, D)


def kernel(x, wq, wk, wv, wo, **_kw):
    x = np.asarray(x, np.float32)
    wq = np.asarray(wq, np.float32)
    wk = np.asarray(wk, np.float32)
    wv = np.asarray(wv, np.float32)
    wo = np.asarray(wo, np.float32)

    if "nc" not in _CACHE:
        _CACHE["nc"] = _build_program()
    nc = _CACHE["nc"]
    in_maps = _host_inputs(x, wq, wk, wv, wo)
    res = run_bass_kernel_spmd(nc, in_maps, core_ids=list(range(NCORES)))
    _CACHE["last_result"] = res
    return _assemble(res.results)


def run_traced(x, wq, wk, wv, wo, tmpdir=None):
    """Like kernel() but with NTFF tracing; returns (out, BassKernelResults)."""
    if "nc" not in _CACHE:
        _CACHE["nc"] = _build_program()
    nc = _CACHE["nc"]
    in_maps = _host_inputs(
        np.asarray(x, np.float32), np.asarray(wq, np.float32),
        np.asarray(wk, np.float32), np.asarray(wv, np.float32),
        np.asarray(wo, np.float32),
    )
    res = run_bass_kernel_spmd(
        nc, in_maps, core_ids=list(range(NCORES)), trace=True, tmpdir=tmpdir
    )
    return _assemble(res.results), res


# revision 24
# speedup vs baseline: 1.0351x; 1.0351x over previous
"""Trainium2 Bass kernel for chunked-local GQA attention (nn_Attention_12266426597578).

Full-input contract: kernel(**inputs) takes the unsharded numpy inputs
(x [1,4096,4096], wq [4096,4096], wk [1024,4096], wv [1024,4096],
wo [4096,4096]) and returns the full output [1,4096,4096].

Sharding (8 cores): 2 chunk-groups (2 attention chunks of 1024 tokens each)
x 4 head-groups (8 q-heads / 2 kv-heads each). Each core computes a partial
y^T [D, 2048] for its token range; the host sums the 4 head-group partials
per token range and concatenates.

Device-side layout: transposed (feature dim on SBUF partitions, tokens on
the free axis) so QKV projections, RoPE (128x128 pair-rotation matmul),
RMSNorm reductions (ones-vector matmuls), scores^T, softmax denominators
and PV all map onto the PE array without on-device transposition of x or
the weights (the host pre-transposes).

This version software-pipelines the per-chunk phases across the two chunks
so the PE array never waits on the ACT-bound attention phase:

    P0(+post0) -> A0 (x) P1raw -> post1 (x) W0[head] -> A1 (x) W0[rest] -> W1

where (x) denotes instruction-level interleaving driven by generators.
Other changes vs the phase-serial baseline: score pairs land in a single
2-bank PSUM tile so each j-block needs ONE exp ACTIVATE; row-replication
(q-norm rows, softmax reciprocal rows) is done with partition-broadcast
DMAs instead of PE matmuls; wo outputs are DMA'd to DRAM directly from
PSUM; the QKV projection runs in 2 d-passes (NQPASS=2) with bf16 partial
sums to halve the resident x working set.
"""

import sys

sys.path.insert(0, "/opt/trn_rl_repo")

import numpy as np
import ml_dtypes
from contextlib import ExitStack

import concourse.bass as bass  # noqa: F401
import concourse.tile as tile
from concourse import bacc, mybir
from concourse.bass_utils import run_bass_kernel_spmd
import concourse.bacc as _bacc_mod
from concourse.hw_specs import get_activation_tables as _gat_orig


def _gat_combined(arch):
    # Restrict the ACT table-load pass to the one set that holds square,
    # ln, exp and copy together, so the kernel never reloads ACT tables.
    t = _gat_orig(arch)
    return {
        k: (v if k == "natural_log_exp_and_others" else set())
        for k, v in t.items()
    }


_bacc_mod.get_activation_tables = _gat_combined

# Problem constants (hardcoded per contract)
S, D = 4096, 4096
H, KVH, HD = 32, 8, 128
CHUNK = 1024
EPS = 1e-5
THETA = 500000.0
ISQ_HD = 1.0 / np.sqrt(np.float32(HD))

# Sharding
NCORES = 8
CG = 2  # chunk groups (token split)
HG = 4  # head groups
TOK = S // CG  # 2048 tokens per core
NCH = TOK // CHUNK  # 2 chunks per core
QH = H // HG  # 8 q heads per core
KH = KVH // HG  # 2 kv heads per core
QO = QH * HD  # 1024
KO = KH * HD  # 256
QKVO = QO + 2 * KO  # 1536
NOT = QKVO // 128  # 12 o-tiles: 0..1 k, 2..3 v, 4..11 q
NB = CHUNK // 128  # 8 blocks of 128 tokens per chunk

f32 = mybir.dt.float32
f32r = mybir.dt.float32r
bf16 = mybir.dt.bfloat16

NPASS = 2  # d-accumulation passes (bf16 SBUF partial sums)
DPP = D // 128 // NPASS  # d-tiles per pass (16)
_CACHE = {}

SDT = bf16
SNP = ml_dtypes.bfloat16

_DONE = object()


def _drive(gen, n):
    """Advance a generator by n yield-steps; returns False when exhausted."""
    for _ in range(n):
        if next(gen, _DONE) is _DONE:
            return False
    return True


def _drain(gen):
    for _ in gen:
        pass


class Ctx:
    """Shared emission context for one core program."""

    def __init__(self, nc, sb, ps, consts, dram):
        self.nc = nc
        self.sb = sb
        self.ps = ps
        self.c = consts
        self.xT, self.wqkvT, self.woT, self.yT = dram
        # per-chunk state
        self.qk = {}       # (ci, ot) -> raw qkv tile (bf16)
        self.fin = {}      # (ci, ot) -> rope+norm'd tile (bf16) for k/q
        self.vtok = {}     # (ci, kvh) -> v transposed to token-major
        self.rkln = {}     # (ci, kvh) -> ln(mean sq + eps) per key block
        self.at = {}       # (ci, h) -> attention output tile


# ---------------------------------------------------------------------------
# Phase 1: raw QKV projection (no post-processing)
# ---------------------------------------------------------------------------

def gen_proj(g, ci, inline_post=False):
    """Raw projection matmuls for chunk ci. Yields after every 4-dk group
    (8 matmuls ~= 4096 cols ~= 1.7us of PE). If inline_post, runs post(ot)
    for each o-tile as its accumulation completes (pass 1)."""
    nc, sb, ps = g.nc, g.sb, g.ps
    s0 = ci * CHUNK
    xts = [None] * (NPASS * DPP)
    post = gen_post(g, ci) if inline_post else None

    def xtile(dk):
        if xts[dk] is None:
            xt = sb.tile([128, CHUNK], SDT, name=f"xt_{ci}_{dk}", tag="xt",
                         bufs=DPP + 3)
            nc.gpsimd.dma_start(
                xt[:], g.xT[dk * 128 : (dk + 1) * 128, s0 : s0 + CHUNK]
            )
            xts[dk] = xt
        return xts[dk]

    for ot in range(NOT):
        g.qk[(ci, ot)] = sb.tile([128, CHUNK], bf16, name=f"qkv_{ci}_{ot}",
                                 tag="qk", bufs=NOT + 2)

    wts = {}

    def wload(p, ot):
        if p >= NPASS:
            return None
        if ot >= NOT:
            p, ot = p + 1, 0
            if p >= NPASS:
                return None
        if (p, ot) not in wts:
            wt = sb.tile([128, DPP * 128], SDT, name=f"w_{ci}_{p}_{ot}",
                         tag="w", bufs=4)
            weng = nc.sync if ot % 2 == 0 else nc.gpsimd
            weng.dma_start(wt[:], g.wqkvT[ot, :, p * DPP : (p + 1) * DPP, :])
            wts[(p, ot)] = wt
        return wts[(p, ot)]

    for p in range(NPASS):
        for ot in range(NOT):
            wt = wload(p, ot)
            wload(p, ot + 1)  # prefetch the next weight block one og ahead
            if ot == 0:
                # issue the whole pass's x DMAs before any matmul so the
                # PE never waits mid-stream
                for dkk in range(DPP):
                    xtile(p * DPP + dkk)
                yield
            # sh-outer: finish one half-accumulator fully before starting the
            # next, so its PSUM->SBUF copy has a whole half-og of slack before
            # the ring slot is needed again.
            for sh in range(2):
                acc = ps.tile([128, 512], f32, name=f"pj_{ci}_{p}_{ot}_{sh}",
                              tag="pacc", bufs=3)
                for gg in range(DPP // 4):
                    for dkk in range(gg * 4, gg * 4 + 4):
                        xtl = xtile(p * DPP + dkk)
                        nc.tensor.matmul(
                            acc[:],
                            lhsT=wt[:, dkk * 128 : (dkk + 1) * 128],
                            rhs=xtl[:, sh * 512 : (sh + 1) * 512],
                            start=(dkk == 0),
                            stop=(dkk == DPP - 1),
                        )
                    yield
                dst = g.qk[(ci, ot)][:, sh * 512 : (sh + 1) * 512]
                if p == 0:
                    nc.vector.tensor_copy(dst, acc[:])
                else:
                    nc.vector.tensor_tensor(
                        dst, dst, acc[:], mybir.AluOpType.add
                    )
            if inline_post and p == NPASS - 1 and ot >= 1:
                # lag post by one o-tile so its ACT chains never stall the PE
                _drive(post, 1)
    if inline_post:
        _drain(post)


# ---------------------------------------------------------------------------
# Post-processing: RoPE + RMSNorm (q/k), transpose (v)
# ---------------------------------------------------------------------------

def gen_post(g, ci):
    """Per o-tile RoPE/RMSNorm/transpose. Yields once per o-tile."""
    nc, sb, ps = g.nc, g.sb, g.ps
    c = g.c
    s0 = ci * CHUNK
    pending = []

    for ot in range(NOT):
        flush = pending[:]
        pending.clear()
        raw = g.qk[(ci, ot)]
        if 2 <= ot < 4:  # v: PE transpose to token-major, bf16
            kvh = ot - 2
            vt = sb.tile([128, CHUNK], bf16, name=f"v_{ci}_{kvh}", tag="v",
                         bufs=5)
            for b in range(NB):
                tp = ps.tile([128, 128], bf16, name=f"vtp_{ci}_{kvh}_{b}",
                             tag="pv", bufs=2)
                nc.tensor.matmul(
                    tp[:],
                    lhsT=raw[:, b * 128 : (b + 1) * 128],
                    rhs=c["identb"][:],
                    is_transpose=True,
                )
                nc.vector.tensor_copy(vt[:, b * 128 : (b + 1) * 128], tp[:])
            g.vtok[(ci, kvh)] = vt
            for f in flush:
                f()
            yield
            continue

        is_q = ot >= 4
        # sumsq over head-dim partitions (RoPE-invariant -> use raw tile)
        sqh = []
        for sh in range(2):
            t = sb.tile([128, 512], bf16, name=f"sq_{ci}_{ot}_{sh}", tag="sq",
                        bufs=2)
            nc.scalar.activation(t[:], raw[:, sh * 512 : (sh + 1) * 512],
                                 mybir.ActivationFunctionType.Square)
            sqh.append(t)
        smt = ps.tile([128, 512], f32, name=f"ssq_{ci}_{ot}", tag="sm", bufs=1)
        if is_q:
            h = ot - 4
            # rows 0 / 32 of one PSUM bank hold the two 512-halves
            nc.tensor.matmul(smt[0:1, :], lhsT=c["onesb"][:], rhs=sqh[0][:])
            nc.tensor.matmul(smt[32:33, :], lhsT=c["onesb"][:], rhs=sqh[1][:])
            lnr = sb.tile([1, CHUNK], f32, name=f"lnq_{ci}_{ot}", tag="rowq",
                          bufs=2)
            nc.scalar.activation(
                lnr[:, 0:512], smt[0:1, :], mybir.ActivationFunctionType.Ln,
                bias=c["beps"][0:1, :], scale=float(1.0 / HD)
            )
            nc.scalar.activation(
                lnr[:, 512:1024], smt[32:33, :],
                mybir.ActivationFunctionType.Ln,
                bias=c["beps"][0:1, :], scale=float(1.0 / HD)
            )
            rows = sb.tile([1, CHUNK], bf16, name=f"rqr_{ci}_{ot}", tag="rowq",
                           bufs=2)
            nc.scalar.activation(
                rows[:], lnr[:], mybir.ActivationFunctionType.Exp, scale=-0.5
            )
            # replicate rows to all partitions via DMA broadcast
            bc = sb.tile([128, CHUNK], bf16, name=f"bcq_{ci}_{ot}", tag="bc",
                         bufs=3)
            nc.gpsimd.partition_broadcast(bc[:], rows[:])
        else:
            kvh = ot
            for b in range(NB):
                nc.tensor.matmul(
                    smt[:, b : b + 1],
                    lhsT=sqh[b // 4][:, (b % 4) * 128 : (b % 4 + 1) * 128],
                    rhs=c["onesb"][:],
                )
            # ln(mean+eps); rks = exp(-0.5*ln + ln(1/sqrt(HD))) emitted at the
            # attention boundary (same EXP table as the softmax).
            lnk = sb.tile([128, NB], f32, name=f"lnk_{ci}_{kvh}", tag="rk",
                          bufs=10)
            nc.scalar.activation(
                lnk[:], smt[:, 0:NB], mybir.ActivationFunctionType.Ln,
                bias=c["beps"][:, :], scale=float(1.0 / HD)
            )
            g.rkln[(ci, kvh)] = lnk

        # RoPE (fp32 math): fin <- (raw*cos + (R @ raw)*sin) [*1/rms for q]
        fin = sb.tile([128, CHUNK], bf16, name=f"fin_{ci}_{ot}", tag="qb",
                      bufs=14)
        for sh in range(2):
            rot = ps.tile([128, 512], f32, name=f"rot_{ci}_{ot}_{sh}",
                          tag="pv", bufs=2)
            nc.tensor.matmul(
                rot[:], lhsT=c["rmat"][:], rhs=raw[:, sh * 512 : (sh + 1) * 512]
            )
            nc.vector.tensor_tensor(
                rot[:], rot[:],
                g.c["sintab"][:, s0 + sh * 512 : s0 + (sh + 1) * 512],
                mybir.AluOpType.mult,
            )
            nc.vector.tensor_tensor(
                raw[:, sh * 512 : (sh + 1) * 512],
                raw[:, sh * 512 : (sh + 1) * 512],
                g.c["costab"][:, s0 + sh * 512 : s0 + (sh + 1) * 512],
                mybir.AluOpType.mult,
            )
            nc.vector.tensor_tensor(
                fin[:, sh * 512 : (sh + 1) * 512],
                raw[:, sh * 512 : (sh + 1) * 512],
                rot[:],
                mybir.AluOpType.add,
            )
        if is_q:
            def fin_q(fin=fin, bc=bc):
                nc.vector.tensor_tensor(
                    fin[:], fin[:], bc[:], mybir.AluOpType.mult
                )
            pending.append(fin_q)
        g.fin[(ci, ot)] = fin
        for f in flush:
            f()
        yield
    for f in pending:
        f()


# ---------------------------------------------------------------------------
# Phase 2: attention (block-causal within chunk)
# ---------------------------------------------------------------------------

def gen_attn(g, ci):
    """Yields once per (head, j-block) plus once per head tail."""
    nc, sb, ps = g.nc, g.sb, g.ps
    c = g.c
    hpending = []
    for kvh in range(KH):
        kf = g.fin[(ci, kvh)]
        vt = g.vtok[(ci, kvh)]
        # rks = exp(-0.5*ln(mean+eps) + ln(1/sqrt(HD))) = 1/(rms*sqrt(HD))
        rks = sb.tile([128, NB], f32, name=f"rks_{ci}_{kvh}", tag="rk",
                      bufs=10)
        nc.scalar.activation(
            rks[:], g.rkln[(ci, kvh)][:], mybir.ActivationFunctionType.Exp,
            bias=c["blniq"][:, :], scale=-0.5,
        )
        for h4 in range(4):
            h = kvh * 4 + h4
            qf = g.fin[(ci, 4 + h)]
            pvA = ps.tile([128, 512], f32, name=f"pvA_{ci}_{h}", tag="pv",
                          bufs=2)
            pvB = ps.tile([128, 512], f32, name=f"pvB_{ci}_{h}", tag="pv",
                          bufs=2)
            # two sum rows packed in one PSUM bank; head parity picks rows
            # (0,32) vs (64,96) so consecutive heads don't serialize.
            smt = ps.tile([128, 512], f32, name=f"sm_{ci}_{h}", tag="sm",
                          bufs=1) if h % 2 == 0 else None
            if smt is not None:
                g._smt = smt
            smt = g._smt
            ra = (h % 2) * 64
            rb = ra + 32
            at = sb.tile([128, CHUNK], SDT, name=f"attn_{ci}_{h}", tag="attn",
                         bufs=18)
            for j in range(NB):
                w = CHUNK - j * 128
                c0 = j * 128
                lenA = 512 - c0 if j < 4 else 0
                b0 = max(512, c0)
                sc = ps.tile([128, CHUNK], f32, name=f"sc_{ci}_{h}_{j}",
                             tag="sc", bufs=1)
                pt = sb.tile([128, CHUNK], bf16, name=f"pt_{ci}_{h}_{j}",
                             tag="pt", bufs=3)
                if lenA > 0:
                    nc.tensor.matmul(
                        sc[:, c0:512],
                        lhsT=kf[:, c0 : c0 + 128],
                        rhs=qf[:, c0:512],
                    )
                    nc.tensor.matmul(
                        sc[:, 512:CHUNK],
                        lhsT=kf[:, c0 : c0 + 128],
                        rhs=qf[:, 512:CHUNK],
                    )
                    # one exp over the contiguous [c0:1024) score range
                    nc.scalar.activation(
                        pt[:, 0:w], sc[:, c0:CHUNK],
                        mybir.ActivationFunctionType.Exp,
                        scale=rks[:, j : j + 1],
                    )
                else:
                    nc.tensor.matmul(
                        sc[:, 0:w],
                        lhsT=kf[:, c0 : c0 + 128],
                        rhs=qf[:, b0:CHUNK],
                    )
                    nc.scalar.activation(
                        pt[:, 0:w], sc[:, 0:w],
                        mybir.ActivationFunctionType.Exp,
                        scale=rks[:, j : j + 1],
                    )
                yield  # interleave point: ACT computes exp meanwhile
                nc.vector.tensor_tensor(
                    pt[:, 0:128], pt[:, 0:128], c["mask"][:],
                    mybir.AluOpType.mult
                )
                if lenA > 0:
                    nc.tensor.matmul(
                        smt[ra : ra + 1, c0:512], lhsT=c["onesb"][:],
                        rhs=pt[:, 0:lenA],
                        start=(j == 0), stop=(j == 3),
                        tile_position=(0, ra),
                    )
                nc.tensor.matmul(
                    smt[rb : rb + 1, b0 - 512 : 512], lhsT=c["onesb"][:],
                    rhs=pt[:, b0 - c0 : w],
                    start=(j == 0), stop=(j == NB - 1),
                    tile_position=(0, rb),
                )
                if lenA > 0:
                    nc.tensor.matmul(
                        pvA[:, c0:512],
                        lhsT=vt[:, c0 : c0 + 128],
                        rhs=pt[:, 0:lenA],
                        start=(j == 0), stop=(j == 3),
                    )
                    if j == 3:
                        # evacuate the finished A half right away to free the
                        # PSUM bank for the next head
                        nc.vector.tensor_copy(at[:, 0:512], pvA[:])
                nc.tensor.matmul(
                    pvB[:, b0 - 512 : 512],
                    lhsT=vt[:, c0 : c0 + 128],
                    rhs=pt[:, b0 - c0 : w],
                    start=(j == 0), stop=(j == NB - 1),
                )
                if j == NB - 1:
                    nc.vector.tensor_copy(at[:, 512:1024], pvB[:])
            # denominators: 1/sum = exp(-ln(sum)); broadcast by DMA; the
            # normalize multiply is deferred one head so PE never stalls.
            lns = sb.tile([1, CHUNK], f32, name=f"lns_{ci}_{h}", tag="rowq",
                          bufs=2)
            nc.scalar.activation(lns[:, 0:512], smt[ra : ra + 1, :],
                                 mybir.ActivationFunctionType.Ln)
            nc.scalar.activation(lns[:, 512:1024], smt[rb : rb + 1, :],
                                 mybir.ActivationFunctionType.Ln)
            srows = sb.tile([1, CHUNK], bf16, name=f"srows_{ci}_{h}",
                            tag="srow", bufs=2)
            nc.scalar.activation(
                srows[:], lns[:], mybir.ActivationFunctionType.Exp, scale=-1.0
            )
            bcs = sb.tile([128, CHUNK], bf16, name=f"bcs_{ci}_{h}", tag="bc",
                          bufs=3)
            nc.gpsimd.partition_broadcast(bcs[:], srows[:])
            flush = hpending[:]
            hpending.clear()
            for f in flush:
                f()

            def fin_head(at=at, bcs=bcs):
                nc.vector.tensor_tensor(
                    at[:], at[:], bcs[:], mybir.AluOpType.mult
                )
            hpending.append(fin_head)
            g.at[(ci, h)] = at
            yield
    for f in hpending:
        f()


# ---------------------------------------------------------------------------
# Phase 3: output projection (y DMA'd straight from PSUM)
# ---------------------------------------------------------------------------

def gen_wo(g, ci):
    """Yields 4x per dd tile (after every 4 accumulation matmuls)."""
    nc, sb, ps = g.nc, g.sb, g.ps
    s0 = ci * CHUNK
    wobs = {}

    def wob_load(dd):
        if dd not in wobs and dd < 32:
            t = sb.tile([128, QO], SDT, name=f"wo_{ci}_{dd}", tag="wo", bufs=4)
            nc.gpsimd.dma_start(t[:], g.woT[dd])
            wobs[dd] = t
        return wobs.get(dd)

    for dd in range(32):
        wob = wob_load(dd)
        wob_load(dd + 1)  # double-buffer the next weight block
        ysb = sb.tile([128, CHUNK], f32, name=f"ysb_{ci}_{dd}", tag="y", bufs=2)
        for sh in range(2):
            yps = ps.tile([128, 512], f32, name=f"y_{ci}_{dd}_{sh}",
                          tag="pacc", bufs=3)
            for h in range(QH):
                nc.tensor.matmul(
                    yps[:],
                    lhsT=wob[:, h * 128 : (h + 1) * 128],
                    rhs=g.at[(ci, h)][:, sh * 512 : (sh + 1) * 512],
                    start=(h == 0), stop=(h == QH - 1),
                )
                if h % 4 == 3:
                    yield
            nc.vector.tensor_copy(
                ysb[:, sh * 512 : (sh + 1) * 512], yps[:]
            )
        nc.sync.dma_start(
            g.yT[dd * 128 : (dd + 1) * 128, s0 : s0 + CHUNK], ysb[:]
        )


def _interleave(main, aux, n_main, n_aux):
    """Drive main to exhaustion, advancing aux proportionally (n_aux aux
    steps spread over n_main main steps)."""
    done_aux = 0
    i = 0
    while True:
        if next(main, _DONE) is _DONE:
            break
        i += 1
        tgt = min(n_aux, i * n_aux // max(1, n_main))
        if tgt > done_aux:
            if _drive(aux, tgt - done_aux):
                done_aux = tgt
            else:
                done_aux = n_aux


def _build_program():
    nc = bacc.Bacc(
        "TRN2",
        target_bir_lowering=False,
        debug=False,
        enable_asserts=False,
        num_devices=NCORES,
    )
    xT = nc.dram_tensor("xT", [D, TOK], SDT, kind="ExternalInput").ap()
    # [ot, p, dk, c]: per (ot, pass) slice is one contiguous-per-partition DMA
    wqkvT = nc.dram_tensor(
        "wqkvT", [NOT, 128, D // 128, 128], SDT, kind="ExternalInput"
    ).ap()
    # per-dd-block tiled layout: woT[dd, p, h*128+c] = wo[dd*128+c, hg*QO + h*128+p]
    woT = nc.dram_tensor("woT", [D // 128, 128, QO], SDT, kind="ExternalInput").ap()
    costab = nc.dram_tensor("costab", [HD, TOK], bf16, kind="ExternalInput").ap()
    sintab = nc.dram_tensor("sintab", [HD, TOK], bf16, kind="ExternalInput").ap()
    rmat = nc.dram_tensor("rmat", [128, 128], bf16, kind="ExternalInput").ap()
    identb = nc.dram_tensor("identb", [128, 128], bf16, kind="ExternalInput").ap()
    mask = nc.dram_tensor("mask", [128, 128], bf16, kind="ExternalInput").ap()
    onesb = nc.dram_tensor("onesb", [128, 1], bf16, kind="ExternalInput").ap()
    beps = nc.dram_tensor("beps", [128, 1], f32, kind="ExternalInput").ap()
    blniq = nc.dram_tensor("blniq", [128, 1], f32, kind="ExternalInput").ap()
    yT = nc.dram_tensor("yT", [D, TOK], f32, kind="ExternalOutput").ap()

    with tile.TileContext(nc) as tc, ExitStack() as ctx:
        ctx.enter_context(nc.allow_low_precision(reason="bf16 attention operands"))
        sb = ctx.enter_context(tc.tile_pool(name="sb", bufs=1))
        ps = ctx.enter_context(tc.tile_pool(name="ps", bufs=1, space="PSUM"))
        cp = ctx.enter_context(tc.tile_pool(name="cp", bufs=1))

        consts = {}
        for nm, ap_, shape, dt_ in (
            ("costab", costab, [HD, TOK], bf16),
            ("sintab", sintab, [HD, TOK], bf16),
            ("rmat", rmat, [128, 128], bf16),
            ("identb", identb, [128, 128], bf16),
            ("mask", mask, [128, 128], bf16),
            ("onesb", onesb, [128, 1], bf16),
            ("beps", beps, [128, 1], f32),
            ("blniq", blniq, [128, 1], f32),
        ):
            t = cp.tile(shape, dt_, name=f"c_{nm}")
            nc.scalar.dma_start(t[:], ap_[:])
            consts[nm] = t

        g = Ctx(nc, sb, ps, consts, (xT, wqkvT, woT, yT))

        # P0 with inline post
        _drain(gen_proj(g, 0, inline_post=True))
        # A0 (x) P1raw: ~72 attn steps over 194 proj yield units
        gp1 = gen_proj(g, 1, inline_post=False)
        _interleave(gen_attn(g, 0), gp1, n_main=72, n_aux=194)
        _drain(gp1)
        # post1 (x) first dds of W0: 12 post steps over 44 wo units (11 dd)
        gw0 = gen_wo(g, 0)
        _drive(gw0, 4)  # first dd fully in flight before post1's chains
        _interleave(gen_post(g, 1), gw0, n_main=12, n_aux=52)
        # A1 (x) rest of W0 (~72 wo units over 72 attn steps)
        _interleave(gen_attn(g, 1), gw0, n_main=72, n_aux=72)
        _drain(gw0)
        # W1
        _drain(gen_wo(g, 1))

    nc.compile()
    return nc


def _host_inputs(x, wq, wk, wv, wo):
    xf = np.ascontiguousarray(x.reshape(S, D).T.astype(SNP))  # [D, S]
    half = HD // 2
    inv_freq = (1.0 / (THETA ** (np.arange(0, half, dtype=np.float32) / half))).astype(
        np.float32
    )
    ang = np.arange(S, dtype=np.float32)[:, None] * inv_freq[None, :]
    cos = np.cos(ang).astype(np.float32)
    sin = np.sin(ang).astype(np.float32)
    costab = np.empty((HD, S), np.float32)
    sintab = np.empty((HD, S), np.float32)
    costab[0::2, :] = cos.T
    costab[1::2, :] = cos.T
    sintab[0::2, :] = sin.T
    sintab[1::2, :] = sin.T
    costab = costab.astype(ml_dtypes.bfloat16)
    sintab = sintab.astype(ml_dtypes.bfloat16)

    rmat = np.zeros((128, 128), ml_dtypes.bfloat16)
    for i in range(64):
        rmat[2 * i + 1, 2 * i] = -1.0
        rmat[2 * i, 2 * i + 1] = 1.0
    identb = np.eye(128, dtype=ml_dtypes.bfloat16)
    mask = np.triu(np.ones((128, 128), np.float32)).astype(ml_dtypes.bfloat16)
    onesb = np.ones((128, 1), ml_dtypes.bfloat16)
    beps = np.full((128, 1), EPS, np.float32)
    blniq = np.full((128, 1), np.log(ISQ_HD), np.float32)

    xT_cg = [np.ascontiguousarray(xf[:, cg * TOK : (cg + 1) * TOK]) for cg in range(CG)]
    cos_cg = [np.ascontiguousarray(costab[:, cg * TOK : (cg + 1) * TOK]) for cg in range(CG)]
    sin_cg = [np.ascontiguousarray(sintab[:, cg * TOK : (cg + 1) * TOK]) for cg in range(CG)]
    wqkvT_hg = []
    woT_hg = []
    for hg in range(HG):
        wq_c = wq[hg * QO : (hg + 1) * QO]
        wk_c = wk[hg * KO : (hg + 1) * KO]
        wv_c = wv[hg * KO : (hg + 1) * KO]
        # o-tile order on device: [k0 k1 v0 v1 q0..q7]
        wcat = np.concatenate([wk_c, wv_c, wq_c], 0)  # [QKVO, D]
        # wqkvT[ot, p, dk, c] = wcat[ot*128 + c?, ...]: lhsT tile for (ot, dk)
        # is w[dk*128+p(part), ot*128+c(free)] = wcat[ot*128+c, dk*128+p].
        wr = wcat.T.reshape(D // 128, 128, NOT, 128)  # [dk, p, ot, c]
        wr = wr.transpose(2, 1, 0, 3)  # [ot, p, dk, c]
        wqkvT_hg.append(np.ascontiguousarray(wr.astype(SNP)))
        wo_c = wo[:, hg * QO : (hg + 1) * QO]  # [D, QO]
        woH = wo_c.reshape(D // 128, 128, QH, 128).transpose(0, 3, 2, 1)  # [dd, p, hb, c]
        woT_hg.append(np.ascontiguousarray(woH.reshape(D // 128, 128, QO).astype(SNP)))

    in_maps = []
    for c in range(NCORES):
        cg, hg = c // HG, c % HG
        in_maps.append(
            {
                "xT": xT_cg[cg],
                "wqkvT": wqkvT_hg[hg],
                "woT": woT_hg[hg],
                "costab": cos_cg[cg],
                "sintab": sin_cg[cg],
                "rmat": rmat,
                "identb": identb,
                "mask": mask,
                "onesb": onesb,
                "beps": beps,
                "blniq": blniq,
            }
        )
    return in_maps


def _assemble(results):
    y = np.empty((S, D), np.float32)
    for cg in range(CG):
        acc = results[cg * HG]["yT"].astype(np.float32)
        for hg in range(1, HG):
            acc = acc + results[cg * HG + hg]["yT"]
        y[cg * TOK : (cg + 1) * TOK, :] = acc.T
    return y.reshape(1, S, D)


def kernel(x, wq, wk, wv, wo, **_kw):
    x = np.asarray(x, np.float32)
    wq = np.asarray(wq, np.float32)
    wk = np.asarray(wk, np.float32)
    wv = np.asarray(wv, np.float32)
    wo = np.asarray(wo, np.float32)

    if "nc" not in _CACHE:
        _CACHE["nc"] = _build_program()
    nc = _CACHE["nc"]
    in_maps = _host_inputs(x, wq, wk, wv, wo)
    res = run_bass_kernel_spmd(nc, in_maps, core_ids=list(range(NCORES)))
    _CACHE["last_result"] = res
    return _assemble(res.results)


def run_traced(x, wq, wk, wv, wo, tmpdir=None):
    """Like kernel() but with NTFF tracing; returns (out, BassKernelResults)."""
    if "nc" not in _CACHE:
        _CACHE["nc"] = _build_program()
    nc = _CACHE["nc"]
    in_maps = _host_inputs(
        np.asarray(x, np.float32), np.asarray(wq, np.float32),
        np.asarray(wk, np.float32), np.asarray(wv, np.float32),
        np.asarray(wo, np.float32),
    )
    res = run_bass_kernel_spmd(
        nc, in_maps, core_ids=list(range(NCORES)), trace=True, tmpdir=tmpdir
    )
    return _assemble(res.results), res


# revision 28
# speedup vs baseline: 1.0360x; 1.0008x over previous
"""Trainium2 Bass kernel for chunked-local GQA attention (nn_Attention_12266426597578).

Full-input contract: kernel(**inputs) takes the unsharded numpy inputs
(x [1,4096,4096], wq [4096,4096], wk [1024,4096], wv [1024,4096],
wo [4096,4096]) and returns the full output [1,4096,4096].

Sharding (8 cores): 2 chunk-groups (2 attention chunks of 1024 tokens each)
x 4 head-groups (8 q-heads / 2 kv-heads each). Each core computes a partial
y^T [D, 2048] for its token range; the host sums the 4 head-group partials
per token range and concatenates.

Device-side layout: transposed (feature dim on SBUF partitions, tokens on
the free axis) so QKV projections, RoPE (128x128 pair-rotation matmul),
RMSNorm reductions (ones-vector matmuls), scores^T, softmax denominators
and PV all map onto the PE array without on-device transposition of x or
the weights (the host pre-transposes).

This version software-pipelines the per-chunk phases across the two chunks
so the PE array never waits on the ACT-bound attention phase:

    P0(+post0) -> A0 (x) P1raw -> post1 (x) W0[head] -> A1 (x) W0[rest] -> W1

where (x) denotes instruction-level interleaving driven by generators.
Other changes vs the phase-serial baseline: score pairs land in a single
2-bank PSUM tile so each j-block needs ONE exp ACTIVATE; row-replication
(q-norm rows, softmax reciprocal rows) is done with partition-broadcast
DMAs instead of PE matmuls; wo outputs are DMA'd to DRAM directly from
PSUM; the QKV projection runs in 2 d-passes (NQPASS=2) with bf16 partial
sums to halve the resident x working set.
"""

import sys

sys.path.insert(0, "/opt/trn_rl_repo")

import numpy as np
import ml_dtypes
from contextlib import ExitStack

import concourse.bass as bass  # noqa: F401
import concourse.tile as tile
from concourse import bacc, mybir
from concourse.bass_utils import run_bass_kernel_spmd
import concourse.bacc as _bacc_mod
from concourse.hw_specs import get_activation_tables as _gat_orig


def _gat_combined(arch):
    # Restrict the ACT table-load pass to the one set that holds square,
    # ln, exp and copy together, so the kernel never reloads ACT tables.
    t = _gat_orig(arch)
    return {
        k: (v if k == "natural_log_exp_and_others" else set())
        for k, v in t.items()
    }


_bacc_mod.get_activation_tables = _gat_combined

# Problem constants (hardcoded per contract)
S, D = 4096, 4096
H, KVH, HD = 32, 8, 128
CHUNK = 1024
EPS = 1e-5
THETA = 500000.0
ISQ_HD = 1.0 / np.sqrt(np.float32(HD))

# Sharding
NCORES = 8
CG = 2  # chunk groups (token split)
HG = 4  # head groups
TOK = S // CG  # 2048 tokens per core
NCH = TOK // CHUNK  # 2 chunks per core
QH = H // HG  # 8 q heads per core
KH = KVH // HG  # 2 kv heads per core
QO = QH * HD  # 1024
KO = KH * HD  # 256
QKVO = QO + 2 * KO  # 1536
NOT = QKVO // 128  # 12 o-tiles: 0..1 k, 2..3 v, 4..11 q
NB = CHUNK // 128  # 8 blocks of 128 tokens per chunk

f32 = mybir.dt.float32
f32r = mybir.dt.float32r
bf16 = mybir.dt.bfloat16

NPASS = 2  # d-accumulation passes (bf16 SBUF partial sums)
DPP = D // 128 // NPASS  # d-tiles per pass (16)
_CACHE = {}

SDT = bf16
SNP = ml_dtypes.bfloat16

_DONE = object()


def _drive(gen, n):
    """Advance a generator by n yield-steps; returns False when exhausted."""
    for _ in range(n):
        if next(gen, _DONE) is _DONE:
            return False
    return True


def _drain(gen):
    for _ in gen:
        pass


class Ctx:
    """Shared emission context for one core program."""

    def __init__(self, nc, sb, ps, consts, dram):
        self.nc = nc
        self.sb = sb
        self.ps = ps
        self.c = consts
        self.xT, self.wqkvT, self.woT, self.yT = dram
        # per-chunk state
        self.qk = {}       # (ci, ot) -> raw qkv tile (bf16)
        self.fin = {}      # (ci, ot) -> rope+norm'd tile (bf16) for k/q
        self.vtok = {}     # (ci, kvh) -> v transposed to token-major
        self.rkln = {}     # (ci, kvh) -> ln(mean sq + eps) per key block
        self.at = {}       # (ci, h) -> attention output tile


# ---------------------------------------------------------------------------
# Phase 1: raw QKV projection (no post-processing)
# ---------------------------------------------------------------------------

def gen_proj(g, ci, inline_post=False):
    """Raw projection matmuls for chunk ci. Yields after every 4-dk group
    (8 matmuls ~= 4096 cols ~= 1.7us of PE). If inline_post, runs post(ot)
    for each o-tile as its accumulation completes (pass 1)."""
    nc, sb, ps = g.nc, g.sb, g.ps
    s0 = ci * CHUNK
    xts = [None] * (NPASS * DPP)
    post = gen_post(g, ci) if inline_post else None

    def xtile(dk):
        if xts[dk] is None:
            xt = sb.tile([128, CHUNK], SDT, name=f"xt_{ci}_{dk}", tag="xt",
                         bufs=DPP + 3)
            nc.gpsimd.dma_start(
                xt[:], g.xT[dk * 128 : (dk + 1) * 128, s0 : s0 + CHUNK]
            )
            xts[dk] = xt
        return xts[dk]

    for ot in range(NOT):
        g.qk[(ci, ot)] = sb.tile([128, CHUNK], bf16, name=f"qkv_{ci}_{ot}",
                                 tag="qk", bufs=NOT + 2)

    wts = {}

    def wload(p, ot):
        if p >= NPASS:
            return None
        if ot >= NOT:
            p, ot = p + 1, 0
            if p >= NPASS:
                return None
        if (p, ot) not in wts:
            wt = sb.tile([128, DPP * 128], SDT, name=f"w_{ci}_{p}_{ot}",
                         tag="w", bufs=4)
            weng = nc.sync if ot % 2 == 0 else nc.gpsimd
            weng.dma_start(wt[:], g.wqkvT[ot, :, p * DPP : (p + 1) * DPP, :])
            wts[(p, ot)] = wt
        return wts[(p, ot)]

    for p in range(NPASS):
        for ot in range(NOT):
            wt = wload(p, ot)
            wload(p, ot + 1)  # prefetch the next weight block one og ahead
            if ot == 0:
                # issue the whole pass's x DMAs before any matmul so the
                # PE never waits mid-stream
                for dkk in range(DPP):
                    xtile(p * DPP + dkk)
                yield
            # sh-outer: finish one half-accumulator fully before starting the
            # next, so its PSUM->SBUF copy has a whole half-og of slack before
            # the ring slot is needed again.
            for sh in range(2):
                acc = ps.tile([128, 512], f32, name=f"pj_{ci}_{p}_{ot}_{sh}",
                              tag="pacc", bufs=3)
                for gg in range(DPP // 4):
                    for dkk in range(gg * 4, gg * 4 + 4):
                        xtl = xtile(p * DPP + dkk)
                        nc.tensor.matmul(
                            acc[:],
                            lhsT=wt[:, dkk * 128 : (dkk + 1) * 128],
                            rhs=xtl[:, sh * 512 : (sh + 1) * 512],
                            start=(dkk == 0),
                            stop=(dkk == DPP - 1),
                        )
                    yield
                dst = g.qk[(ci, ot)][:, sh * 512 : (sh + 1) * 512]
                if p == 0:
                    nc.vector.tensor_copy(dst, acc[:])
                else:
                    nc.vector.tensor_tensor(
                        dst, dst, acc[:], mybir.AluOpType.add
                    )
            if inline_post and p == NPASS - 1 and ot >= 1:
                # lag post by one o-tile so its ACT chains never stall the PE
                _drive(post, 1)
    if inline_post:
        _drain(post)


# ---------------------------------------------------------------------------
# Post-processing: RoPE + RMSNorm (q/k), transpose (v)
# ---------------------------------------------------------------------------

def gen_post(g, ci):
    """Per o-tile RoPE/RMSNorm/transpose. Yields once per o-tile."""
    nc, sb, ps = g.nc, g.sb, g.ps
    c = g.c
    s0 = ci * CHUNK
    pending = []

    for ot in range(NOT):
        flush = pending[:]
        pending.clear()
        raw = g.qk[(ci, ot)]
        if 2 <= ot < 4:  # v: PE transpose to token-major, bf16
            kvh = ot - 2
            vt = sb.tile([128, CHUNK], bf16, name=f"v_{ci}_{kvh}", tag="v",
                         bufs=5)
            for b in range(NB):
                tp = ps.tile([128, 128], bf16, name=f"vtp_{ci}_{kvh}_{b}",
                             tag="pv", bufs=2)
                nc.tensor.matmul(
                    tp[:],
                    lhsT=raw[:, b * 128 : (b + 1) * 128],
                    rhs=c["identb"][:],
                    is_transpose=True,
                )
                nc.vector.tensor_copy(vt[:, b * 128 : (b + 1) * 128], tp[:])
            g.vtok[(ci, kvh)] = vt
            for f in flush:
                f()
            yield
            continue

        is_q = ot >= 4
        # sumsq over head-dim partitions (RoPE-invariant -> use raw tile)
        sqh = []
        for sh in range(2):
            t = sb.tile([128, 512], bf16, name=f"sq_{ci}_{ot}_{sh}", tag="sq",
                        bufs=2)
            nc.scalar.activation(t[:], raw[:, sh * 512 : (sh + 1) * 512],
                                 mybir.ActivationFunctionType.Square)
            sqh.append(t)
        smt = ps.tile([128, 512], f32, name=f"ssq_{ci}_{ot}", tag="sm", bufs=1)
        if is_q:
            h = ot - 4
            # rows 0 / 32 of one PSUM bank hold the two 512-halves
            nc.tensor.matmul(smt[0:1, :], lhsT=c["onesb"][:], rhs=sqh[0][:])
            nc.tensor.matmul(smt[32:33, :], lhsT=c["onesb"][:], rhs=sqh[1][:])
            lnr = sb.tile([1, CHUNK], f32, name=f"lnq_{ci}_{ot}", tag="rowq",
                          bufs=2)
            nc.scalar.activation(
                lnr[:, 0:512], smt[0:1, :], mybir.ActivationFunctionType.Ln,
                bias=c["beps"][0:1, :], scale=float(1.0 / HD)
            )
            nc.scalar.activation(
                lnr[:, 512:1024], smt[32:33, :],
                mybir.ActivationFunctionType.Ln,
                bias=c["beps"][0:1, :], scale=float(1.0 / HD)
            )
            rows = sb.tile([1, CHUNK], bf16, name=f"rqr_{ci}_{ot}", tag="rowq",
                           bufs=2)
            nc.scalar.activation(
                rows[:], lnr[:], mybir.ActivationFunctionType.Exp, scale=-0.5
            )
            # replicate rows to all partitions via DMA broadcast
            bc = sb.tile([128, CHUNK], bf16, name=f"bcq_{ci}_{ot}", tag="bc",
                         bufs=3)
            nc.gpsimd.partition_broadcast(bc[:], rows[:])
        else:
            kvh = ot
            for b in range(NB):
                nc.tensor.matmul(
                    smt[:, b : b + 1],
                    lhsT=sqh[b // 4][:, (b % 4) * 128 : (b % 4 + 1) * 128],
                    rhs=c["onesb"][:],
                )
            # ln(mean+eps); rks = exp(-0.5*ln + ln(1/sqrt(HD))) emitted at the
            # attention boundary (same EXP table as the softmax).
            lnk = sb.tile([128, NB], f32, name=f"lnk_{ci}_{kvh}", tag="rk",
                          bufs=10)
            nc.scalar.activation(
                lnk[:], smt[:, 0:NB], mybir.ActivationFunctionType.Ln,
                bias=c["beps"][:, :], scale=float(1.0 / HD)
            )
            g.rkln[(ci, kvh)] = lnk

        # RoPE (fp32 math): fin <- (raw*cos + (R @ raw)*sin) [*1/rms for q]
        fin = sb.tile([128, CHUNK], bf16, name=f"fin_{ci}_{ot}", tag="qb",
                      bufs=14)
        for sh in range(2):
            rot = ps.tile([128, 512], f32, name=f"rot_{ci}_{ot}_{sh}",
                          tag="pv", bufs=2)
            nc.tensor.matmul(
                rot[:], lhsT=c["rmat"][:], rhs=raw[:, sh * 512 : (sh + 1) * 512]
            )
            nc.vector.tensor_tensor(
                rot[:], rot[:],
                g.c["sintab"][:, s0 + sh * 512 : s0 + (sh + 1) * 512],
                mybir.AluOpType.mult,
            )
            nc.vector.tensor_tensor(
                raw[:, sh * 512 : (sh + 1) * 512],
                raw[:, sh * 512 : (sh + 1) * 512],
                g.c["costab"][:, s0 + sh * 512 : s0 + (sh + 1) * 512],
                mybir.AluOpType.mult,
            )
            nc.vector.tensor_tensor(
                fin[:, sh * 512 : (sh + 1) * 512],
                raw[:, sh * 512 : (sh + 1) * 512],
                rot[:],
                mybir.AluOpType.add,
            )
        if is_q:
            def fin_q(fin=fin, bc=bc):
                nc.vector.tensor_tensor(
                    fin[:], fin[:], bc[:], mybir.AluOpType.mult
                )
            pending.append(fin_q)
        g.fin[(ci, ot)] = fin
        for f in flush:
            f()
        yield
    for f in pending:
        f()


# ---------------------------------------------------------------------------
# Phase 2: attention (block-causal within chunk)
# ---------------------------------------------------------------------------

def gen_attn(g, ci):
    """Yields once per (head, j-block) plus once per head tail."""
    nc, sb, ps = g.nc, g.sb, g.ps
    c = g.c
    hpending = []    # deferred at-normalize closures (bcast long done)
    den_pending = []  # deferred denominator chains for the previous head
    # one PSUM bank of sum rows for the whole phase; head parity picks rows
    # (0,32) vs (64,96) so consecutive heads don't serialize.
    smt = ps.tile([128, 512], f32, name=f"sm_{ci}", tag="sm", bufs=1)
    # rks = exp(-0.5*ln(mean+eps) + ln(1/sqrt(HD))) = 1/(rms*sqrt(HD));
    # both kv heads upfront so no exp ever waits on them mid-phase
    rks_t = {}
    for kvh in range(KH):
        rks_t[kvh] = sb.tile([128, NB], f32, name=f"rks_{ci}_{kvh}", tag="rk",
                             bufs=10)
        nc.scalar.activation(
            rks_t[kvh][:], g.rkln[(ci, kvh)][:],
            mybir.ActivationFunctionType.Exp,
            bias=c["blniq"][:, :], scale=-0.5,
        )
    for kvh in range(KH):
        kf = g.fin[(ci, kvh)]
        vt = g.vtok[(ci, kvh)]
        rks = rks_t[kvh]
        for h4 in range(4):
            h = kvh * 4 + h4
            qf = g.fin[(ci, 4 + h)]
            pvA = ps.tile([128, 512], f32, name=f"pvA_{ci}_{h}", tag="pv",
                          bufs=2)
            pvB = ps.tile([128, 512], f32, name=f"pvB_{ci}_{h}", tag="pv",
                          bufs=2)
            ra = (h % 2) * 64
            rb = ra + 32
            at = sb.tile([128, CHUNK], SDT, name=f"attn_{ci}_{h}", tag="attn",
                         bufs=18)
            for j in range(NB):
                w = CHUNK - j * 128
                c0 = j * 128
                lenA = 512 - c0 if j < 4 else 0
                b0 = max(512, c0)
                sc = ps.tile([128, CHUNK], f32, name=f"sc_{ci}_{h}_{j}",
                             tag="sc", bufs=1)
                pt = sb.tile([128, CHUNK], bf16, name=f"pt_{ci}_{h}_{j}",
                             tag="pt", bufs=3)
                if lenA > 0:
                    nc.tensor.matmul(
                        sc[:, c0:512],
                        lhsT=kf[:, c0 : c0 + 128],
                        rhs=qf[:, c0:512],
                    )
                    nc.tensor.matmul(
                        sc[:, 512:CHUNK],
                        lhsT=kf[:, c0 : c0 + 128],
                        rhs=qf[:, 512:CHUNK],
                    )
                    # one exp over the contiguous [c0:1024) score range
                    nc.scalar.activation(
                        pt[:, 0:w], sc[:, c0:CHUNK],
                        mybir.ActivationFunctionType.Exp,
                        scale=rks[:, j : j + 1],
                    )
                else:
                    nc.tensor.matmul(
                        sc[:, 0:w],
                        lhsT=kf[:, c0 : c0 + 128],
                        rhs=qf[:, b0:CHUNK],
                    )
                    nc.scalar.activation(
                        pt[:, 0:w], sc[:, 0:w],
                        mybir.ActivationFunctionType.Exp,
                        scale=rks[:, j : j + 1],
                    )
                if j == 0:
                    # with this head's first exp already queued on ACT, emit
                    # the previous head's denominator chain (ACT) and the
                    # head-before's at-normalize (DVE) off the critical path
                    fl, dl = hpending[:], den_pending[:]
                    hpending.clear()
                    den_pending.clear()
                    for f in fl:
                        f()
                    for f in dl:
                        f()
                yield  # interleave point: ACT computes exp meanwhile
                nc.vector.tensor_tensor(
                    pt[:, 0:128], pt[:, 0:128], c["mask"][:],
                    mybir.AluOpType.mult
                )
                if lenA > 0:
                    nc.tensor.matmul(
                        smt[ra : ra + 1, c0:512], lhsT=c["onesb"][:],
                        rhs=pt[:, 0:lenA],
                        start=(j == 0), stop=(j == 3),
                        tile_position=(0, ra),
                    )
                nc.tensor.matmul(
                    smt[rb : rb + 1, b0 - 512 : 512], lhsT=c["onesb"][:],
                    rhs=pt[:, b0 - c0 : w],
                    start=(j == 0), stop=(j == NB - 1),
                    tile_position=(0, rb),
                )
                if lenA > 0:
                    nc.tensor.matmul(
                        pvA[:, c0:512],
                        lhsT=vt[:, c0 : c0 + 128],
                        rhs=pt[:, 0:lenA],
                        start=(j == 0), stop=(j == 3),
                    )
                    if j == 3:
                        # evacuate the finished A half right away to free the
                        # PSUM bank for the next head
                        nc.vector.tensor_copy(at[:, 0:512], pvA[:])
                nc.tensor.matmul(
                    pvB[:, b0 - 512 : 512],
                    lhsT=vt[:, c0 : c0 + 128],
                    rhs=pt[:, b0 - c0 : w],
                    start=(j == 0), stop=(j == NB - 1),
                )
                if j == NB - 1:
                    nc.vector.tensor_copy(at[:, 512:1024], pvB[:])
            # denominators: 1/sum = exp(-ln(sum)), both halves in one op via
            # the packed sum rows (partitions ra and ra+32); emitted deferred
            # at the NEXT head's j=0 so these ACT ops never delay an exp.
            def den(h=h, ra=ra, rb=rb, at=at):
                lns = sb.tile([1, CHUNK], f32, name=f"lns_{ci}_{h}",
                              tag="rowq", bufs=2)
                nc.scalar.activation(lns[:, 0:512], smt[ra : ra + 1, :],
                                     mybir.ActivationFunctionType.Ln)
                nc.scalar.activation(lns[:, 512:1024], smt[rb : rb + 1, :],
                                     mybir.ActivationFunctionType.Ln)
                srows = sb.tile([1, CHUNK], bf16, name=f"srows_{ci}_{h}",
                                tag="srow", bufs=2)
                nc.scalar.activation(
                    srows[:], lns[:], mybir.ActivationFunctionType.Exp,
                    scale=-1.0
                )
                bcs = sb.tile([128, CHUNK], bf16, name=f"bcs_{ci}_{h}",
                              tag="bc", bufs=3)
                nc.gpsimd.partition_broadcast(bcs[:], srows[:])

                def fin_head(at=at, bcs=bcs):
                    nc.vector.tensor_tensor(
                        at[:], at[:], bcs[:], mybir.AluOpType.mult
                    )
                hpending.append(fin_head)
            den_pending.append(den)
            g.at[(ci, h)] = at
            yield
    for f in den_pending:
        f()
    for f in hpending:
        f()


# ---------------------------------------------------------------------------
# Phase 3: output projection (y DMA'd straight from PSUM)
# ---------------------------------------------------------------------------

def gen_wo(g, ci):
    """Yields 4x per dd tile (after every 4 accumulation matmuls)."""
    nc, sb, ps = g.nc, g.sb, g.ps
    s0 = ci * CHUNK
    wobs = {}

    def wob_load(dd):
        if dd not in wobs and dd < 32:
            t = sb.tile([128, QO], SDT, name=f"wo_{ci}_{dd}", tag="wo", bufs=4)
            nc.gpsimd.dma_start(t[:], g.woT[dd])
            wobs[dd] = t
        return wobs.get(dd)

    for dd in range(32):
        wob = wob_load(dd)
        wob_load(dd + 1)  # double-buffer the next weight block
        ysb = sb.tile([128, CHUNK], f32, name=f"ysb_{ci}_{dd}", tag="y", bufs=2)
        for sh in range(2):
            yps = ps.tile([128, 512], f32, name=f"y_{ci}_{dd}_{sh}",
                          tag="pacc", bufs=3)
            for h in range(QH):
                nc.tensor.matmul(
                    yps[:],
                    lhsT=wob[:, h * 128 : (h + 1) * 128],
                    rhs=g.at[(ci, h)][:, sh * 512 : (sh + 1) * 512],
                    start=(h == 0), stop=(h == QH - 1),
                )
                if h % 4 == 3:
                    yield
            nc.vector.tensor_copy(
                ysb[:, sh * 512 : (sh + 1) * 512], yps[:]
            )
        nc.sync.dma_start(
            g.yT[dd * 128 : (dd + 1) * 128, s0 : s0 + CHUNK], ysb[:]
        )


def _interleave(main, aux, n_main, n_aux):
    """Drive main to exhaustion, advancing aux proportionally (n_aux aux
    steps spread over n_main main steps)."""
    done_aux = 0
    i = 0
    while True:
        if next(main, _DONE) is _DONE:
            break
        i += 1
        tgt = min(n_aux, i * n_aux // max(1, n_main))
        if tgt > done_aux:
            if _drive(aux, tgt - done_aux):
                done_aux = tgt
            else:
                done_aux = n_aux


def _build_program():
    nc = bacc.Bacc(
        "TRN2",
        target_bir_lowering=False,
        debug=False,
        enable_asserts=False,
        num_devices=NCORES,
    )
    xT = nc.dram_tensor("xT", [D, TOK], SDT, kind="ExternalInput").ap()
    # [ot, p, dk, c]: per (ot, pass) slice is one contiguous-per-partition DMA
    wqkvT = nc.dram_tensor(
        "wqkvT", [NOT, 128, D // 128, 128], SDT, kind="ExternalInput"
    ).ap()
    # per-dd-block tiled layout: woT[dd, p, h*128+c] = wo[dd*128+c, hg*QO + h*128+p]
    woT = nc.dram_tensor("woT", [D // 128, 128, QO], SDT, kind="ExternalInput").ap()
    costab = nc.dram_tensor("costab", [HD, TOK], bf16, kind="ExternalInput").ap()
    sintab = nc.dram_tensor("sintab", [HD, TOK], bf16, kind="ExternalInput").ap()
    rmat = nc.dram_tensor("rmat", [128, 128], bf16, kind="ExternalInput").ap()
    identb = nc.dram_tensor("identb", [128, 128], bf16, kind="ExternalInput").ap()
    mask = nc.dram_tensor("mask", [128, 128], bf16, kind="ExternalInput").ap()
    onesb = nc.dram_tensor("onesb", [128, 1], bf16, kind="ExternalInput").ap()
    beps = nc.dram_tensor("beps", [128, 1], f32, kind="ExternalInput").ap()
    blniq = nc.dram_tensor("blniq", [128, 1], f32, kind="ExternalInput").ap()
    yT = nc.dram_tensor("yT", [D, TOK], f32, kind="ExternalOutput").ap()

    with tile.TileContext(nc) as tc, ExitStack() as ctx:
        ctx.enter_context(nc.allow_low_precision(reason="bf16 attention operands"))
        sb = ctx.enter_context(tc.tile_pool(name="sb", bufs=1))
        ps = ctx.enter_context(tc.tile_pool(name="ps", bufs=1, space="PSUM"))
        cp = ctx.enter_context(tc.tile_pool(name="cp", bufs=1))

        consts = {}
        for nm, ap_, shape, dt_ in (
            ("costab", costab, [HD, TOK], bf16),
            ("sintab", sintab, [HD, TOK], bf16),
            ("rmat", rmat, [128, 128], bf16),
            ("identb", identb, [128, 128], bf16),
            ("mask", mask, [128, 128], bf16),
            ("onesb", onesb, [128, 1], bf16),
            ("beps", beps, [128, 1], f32),
            ("blniq", blniq, [128, 1], f32),
        ):
            t = cp.tile(shape, dt_, name=f"c_{nm}")
            nc.scalar.dma_start(t[:], ap_[:])
            consts[nm] = t

        g = Ctx(nc, sb, ps, consts, (xT, wqkvT, woT, yT))

        # P0 with inline post
        _drain(gen_proj(g, 0, inline_post=True))
        # A0 (x) P1raw: ~72 attn steps over 194 proj yield units
        gp1 = gen_proj(g, 1, inline_post=False)
        _interleave(gen_attn(g, 0), gp1, n_main=72, n_aux=194)
        _drain(gp1)
        # post1 (x) first dds of W0: 12 post steps over 44 wo units (11 dd)
        gw0 = gen_wo(g, 0)
        _drive(gw0, 4)  # first dd fully in flight before post1's chains
        _interleave(gen_post(g, 1), gw0, n_main=12, n_aux=52)
        # A1 (x) rest of W0 (~72 wo units over 72 attn steps)
        _interleave(gen_attn(g, 1), gw0, n_main=72, n_aux=72)
        _drain(gw0)
        # W1
        _drain(gen_wo(g, 1))

    nc.compile()
    return nc


def _host_inputs(x, wq, wk, wv, wo):
    xf = np.ascontiguousarray(x.reshape(S, D).T.astype(SNP))  # [D, S]
    half = HD // 2
    inv_freq = (1.0 / (THETA ** (np.arange(0, half, dtype=np.float32) / half))).astype(
        np.float32
    )
    ang = np.arange(S, dtype=np.float32)[:, None] * inv_freq[None, :]
    cos = np.cos(ang).astype(np.float32)
    sin = np.sin(ang).astype(np.float32)
    costab = np.empty((HD, S), np.float32)
    sintab = np.empty((HD, S), np.float32)
    costab[0::2, :] = cos.T
    costab[1::2, :] = cos.T
    sintab[0::2, :] = sin.T
    sintab[1::2, :] = sin.T
    costab = costab.astype(ml_dtypes.bfloat16)
    sintab = sintab.astype(ml_dtypes.bfloat16)

    rmat = np.zeros((128, 128), ml_dtypes.bfloat16)
    for i in range(64):
        rmat[2 * i + 1, 2 * i] = -1.0
        rmat[2 * i, 2 * i + 1] = 1.0
    identb = np.eye(128, dtype=ml_dtypes.bfloat16)
    mask = np.triu(np.ones((128, 128), np.float32)).astype(ml_dtypes.bfloat16)
    onesb = np.ones((128, 1), ml_dtypes.bfloat16)
    beps = np.full((128, 1), EPS, np.float32)
    blniq = np.full((128, 1), np.log(ISQ_HD), np.float32)

    xT_cg = [np.ascontiguousarray(xf[:, cg * TOK : (cg + 1) * TOK]) for cg in range(CG)]
    cos_cg = [np.ascontiguousarray(costab[:, cg * TOK : (cg + 1) * TOK]) for cg in range(CG)]
    sin_cg = [np.ascontiguousarray(sintab[:, cg * TOK : (cg + 1) * TOK]) for cg in range(CG)]
    wqkvT_hg = []
    woT_hg = []
    for hg in range(HG):
        wq_c = wq[hg * QO : (hg + 1) * QO]
        wk_c = wk[hg * KO : (hg + 1) * KO]
        wv_c = wv[hg * KO : (hg + 1) * KO]
        # o-tile order on device: [k0 k1 v0 v1 q0..q7]
        wcat = np.concatenate([wk_c, wv_c, wq_c], 0)  # [QKVO, D]
        # wqkvT[ot, p, dk, c] = wcat[ot*128 + c?, ...]: lhsT tile for (ot, dk)
        # is w[dk*128+p(part), ot*128+c(free)] = wcat[ot*128+c, dk*128+p].
        wr = wcat.T.reshape(D // 128, 128, NOT, 128)  # [dk, p, ot, c]
        wr = wr.transpose(2, 1, 0, 3)  # [ot, p, dk, c]
        wqkvT_hg.append(np.ascontiguousarray(wr.astype(SNP)))
        wo_c = wo[:, hg * QO : (hg + 1) * QO]  # [D, QO]
        woH = wo_c.reshape(D // 128, 128, QH, 128).transpose(0, 3, 2, 1)  # [dd, p, hb, c]
        woT_hg.append(np.ascontiguousarray(woH.reshape(D // 128, 128, QO).astype(SNP)))

    in_maps = []
    for c in range(NCORES):
        cg, hg = c // HG, c % HG
        in_maps.append(
            {
                "xT": xT_cg[cg],
                "wqkvT": wqkvT_hg[hg],
                "woT": woT_hg[hg],
                "costab": cos_cg[cg],
                "sintab": sin_cg[cg],
                "rmat": rmat,
                "identb": identb,
                "mask": mask,
                "onesb": onesb,
                "beps": beps,
                "blniq": blniq,
            }
        )
    return in_maps


def _assemble(results):
    y = np.empty((S, D), np.float32)
    for cg in range(CG):
        acc = results[cg * HG]["yT"].astype(np.float32)
        for hg in range(1, HG):
            acc = acc + results[cg * HG + hg]["yT"]
        y[cg * TOK : (cg + 1) * TOK, :] = acc.T
    return y.reshape(1, S, D)


def kernel(x, wq, wk, wv, wo, **_kw):
    x = np.asarray(x, np.float32)
    wq = np.asarray(wq, np.float32)
    wk = np.asarray(wk, np.float32)
    wv = np.asarray(wv, np.float32)
    wo = np.asarray(wo, np.float32)

    if "nc" not in _CACHE:
        _CACHE["nc"] = _build_program()
    nc = _CACHE["nc"]
    in_maps = _host_inputs(x, wq, wk, wv, wo)
    res = run_bass_kernel_spmd(nc, in_maps, core_ids=list(range(NCORES)))
    _CACHE["last_result"] = res
    return _assemble(res.results)


def run_traced(x, wq, wk, wv, wo, tmpdir=None):
    """Like kernel() but with NTFF tracing; returns (out, BassKernelResults)."""
    if "nc" not in _CACHE:
        _CACHE["nc"] = _build_program()
    nc = _CACHE["nc"]
    in_maps = _host_inputs(
        np.asarray(x, np.float32), np.asarray(wq, np.float32),
        np.asarray(wk, np.float32), np.asarray(wv, np.float32),
        np.asarray(wo, np.float32),
    )
    res = run_bass_kernel_spmd(
        nc, in_maps, core_ids=list(range(NCORES)), trace=True, tmpdir=tmpdir
    )
    return _assemble(res.results), res


# revision 31
# speedup vs baseline: 1.0416x; 1.0055x over previous
"""Trainium2 Bass kernel for chunked-local GQA attention (nn_Attention_12266426597578).

Full-input contract: kernel(**inputs) takes the unsharded numpy inputs
(x [1,4096,4096], wq [4096,4096], wk [1024,4096], wv [1024,4096],
wo [4096,4096]) and returns the full output [1,4096,4096].

Sharding (8 cores): 2 chunk-groups (2 attention chunks of 1024 tokens each)
x 4 head-groups (8 q-heads / 2 kv-heads each). Each core computes a partial
y^T [D, 2048] for its token range; the host sums the 4 head-group partials
per token range and concatenates.

Device-side layout: transposed (feature dim on SBUF partitions, tokens on
the free axis) so QKV projections, RoPE (128x128 pair-rotation matmul),
RMSNorm reductions (ones-vector matmuls), scores^T, softmax denominators
and PV all map onto the PE array without on-device transposition of x or
the weights (the host pre-transposes).

This version software-pipelines the per-chunk phases across the two chunks
so the PE array never waits on the ACT-bound attention phase:

    P0(+post0) -> A0 (x) P1raw -> post1 (x) W0[head] -> A1 (x) W0[rest] -> W1

where (x) denotes instruction-level interleaving driven by generators.
Other changes vs the phase-serial baseline: score pairs land in a single
2-bank PSUM tile so each j-block needs ONE exp ACTIVATE; row-replication
(q-norm rows, softmax reciprocal rows) is done with partition-broadcast
DMAs instead of PE matmuls; wo outputs are DMA'd to DRAM directly from
PSUM; the QKV projection runs in 2 d-passes (NQPASS=2) with bf16 partial
sums to halve the resident x working set.
"""

import sys

sys.path.insert(0, "/opt/trn_rl_repo")

import numpy as np
import ml_dtypes
from contextlib import ExitStack

import concourse.bass as bass  # noqa: F401
import concourse.tile as tile
from concourse import bacc, mybir
from concourse.bass_utils import run_bass_kernel_spmd
import concourse.bacc as _bacc_mod
from concourse.hw_specs import get_activation_tables as _gat_orig


def _gat_combined(arch):
    # Restrict the ACT table-load pass to the one set that holds square,
    # ln, exp and copy together, so the kernel never reloads ACT tables.
    t = _gat_orig(arch)
    return {
        k: (v if k == "natural_log_exp_and_others" else set())
        for k, v in t.items()
    }


_bacc_mod.get_activation_tables = _gat_combined

# Problem constants (hardcoded per contract)
S, D = 4096, 4096
H, KVH, HD = 32, 8, 128
CHUNK = 1024
EPS = 1e-5
THETA = 500000.0
ISQ_HD = 1.0 / np.sqrt(np.float32(HD))

# Sharding
NCORES = 8
CG = 2  # chunk groups (token split)
HG = 4  # head groups
TOK = S // CG  # 2048 tokens per core
NCH = TOK // CHUNK  # 2 chunks per core
QH = H // HG  # 8 q heads per core
KH = KVH // HG  # 2 kv heads per core
QO = QH * HD  # 1024
KO = KH * HD  # 256
QKVO = QO + 2 * KO  # 1536
NOT = QKVO // 128  # 12 o-tiles: 0..1 k, 2..3 v, 4..11 q
NB = CHUNK // 128  # 8 blocks of 128 tokens per chunk

f32 = mybir.dt.float32
f32r = mybir.dt.float32r
bf16 = mybir.dt.bfloat16

NPASS = 2  # d-accumulation passes (bf16 SBUF partial sums)
DPP = D // 128 // NPASS  # d-tiles per pass (16)
_CACHE = {}

SDT = bf16
SNP = ml_dtypes.bfloat16

_DONE = object()


def _drive(gen, n):
    """Advance a generator by n yield-steps; returns False when exhausted."""
    for _ in range(n):
        if next(gen, _DONE) is _DONE:
            return False
    return True


def _drain(gen):
    for _ in gen:
        pass


class Ctx:
    """Shared emission context for one core program."""

    def __init__(self, nc, sb, ps, consts, dram):
        self.nc = nc
        self.sb = sb
        self.ps = ps
        self.c = consts
        self.xT, self.wqkvT, self.woT, self.yT = dram
        # per-chunk state
        self.qk = {}       # (ci, ot) -> raw qkv tile (bf16)
        self.fin = {}      # (ci, ot) -> rope+norm'd tile (bf16) for k/q
        self.vtok = {}     # (ci, kvh) -> v transposed to token-major
        self.rkln = {}     # (ci, kvh) -> ln(mean sq + eps) per key block
        self.at = {}       # (ci, h) -> attention output tile


# ---------------------------------------------------------------------------
# Phase 1: raw QKV projection (no post-processing)
# ---------------------------------------------------------------------------

def gen_proj(g, ci, inline_post=False):
    """Raw projection matmuls for chunk ci. Yields after every 4-dk group
    (8 matmuls ~= 4096 cols ~= 1.7us of PE). If inline_post, runs post(ot)
    for each o-tile as its accumulation completes (pass 1)."""
    nc, sb, ps = g.nc, g.sb, g.ps
    s0 = ci * CHUNK
    xts = [None] * (NPASS * DPP)
    post = gen_post(g, ci) if inline_post else None

    def xtile(dk):
        if xts[dk] is None:
            xt = sb.tile([128, CHUNK], SDT, name=f"xt_{ci}_{dk}", tag="xt",
                         bufs=DPP + 3)
            nc.gpsimd.dma_start(
                xt[:], g.xT[dk * 128 : (dk + 1) * 128, s0 : s0 + CHUNK]
            )
            xts[dk] = xt
        return xts[dk]

    for ot in range(NOT):
        g.qk[(ci, ot)] = sb.tile([128, CHUNK], bf16, name=f"qkv_{ci}_{ot}",
                                 tag="qk", bufs=NOT + 2)

    wts = {}

    def wload(p, ot):
        if p >= NPASS:
            return None
        if ot >= NOT:
            p, ot = p + 1, 0
            if p >= NPASS:
                return None
        if (p, ot) not in wts:
            wt = sb.tile([128, DPP * 128], SDT, name=f"w_{ci}_{p}_{ot}",
                         tag="w", bufs=4)
            weng = nc.sync if ot % 2 == 0 else nc.gpsimd
            weng.dma_start(wt[:], g.wqkvT[ot, :, p * DPP : (p + 1) * DPP, :])
            wts[(p, ot)] = wt
        return wts[(p, ot)]

    for p in range(NPASS):
        for ot in range(NOT):
            wt = wload(p, ot)
            wload(p, ot + 1)  # prefetch the next weight block one og ahead
            if ot == 0:
                # issue the whole pass's x DMAs before any matmul so the
                # PE never waits mid-stream
                for dkk in range(DPP):
                    xtile(p * DPP + dkk)
                yield
            # sh-outer: finish one half-accumulator fully before starting the
            # next, so its PSUM->SBUF copy has a whole half-og of slack before
            # the ring slot is needed again.
            for sh in range(2):
                acc = ps.tile([128, 512], f32, name=f"pj_{ci}_{p}_{ot}_{sh}",
                              tag="pacc", bufs=3)
                for gg in range(DPP // 4):
                    for dkk in range(gg * 4, gg * 4 + 4):
                        xtl = xtile(p * DPP + dkk)
                        nc.tensor.matmul(
                            acc[:],
                            lhsT=wt[:, dkk * 128 : (dkk + 1) * 128],
                            rhs=xtl[:, sh * 512 : (sh + 1) * 512],
                            start=(dkk == 0),
                            stop=(dkk == DPP - 1),
                        )
                    yield
                dst = g.qk[(ci, ot)][:, sh * 512 : (sh + 1) * 512]
                if p == 0:
                    nc.vector.tensor_copy(dst, acc[:])
                else:
                    nc.vector.tensor_tensor(
                        dst, dst, acc[:], mybir.AluOpType.add
                    )
            if inline_post and p == NPASS - 1 and ot >= 1:
                # lag post by one o-tile so its ACT chains never stall the PE
                _drive(post, 1)
    if inline_post:
        _drain(post)


# ---------------------------------------------------------------------------
# Post-processing: RoPE + RMSNorm (q/k), transpose (v)
# ---------------------------------------------------------------------------

def gen_post(g, ci):
    """Per o-tile RoPE/RMSNorm/transpose. Yields once per o-tile."""
    nc, sb, ps = g.nc, g.sb, g.ps
    c = g.c
    s0 = ci * CHUNK
    pending = []

    for ot in range(NOT):
        flush = pending[:]
        pending.clear()
        raw = g.qk[(ci, ot)]
        if 2 <= ot < 4:  # v: PE transpose to token-major, bf16
            kvh = ot - 2
            vt = sb.tile([128, CHUNK], bf16, name=f"v_{ci}_{kvh}", tag="v",
                         bufs=5)
            for b in range(NB):
                tp = ps.tile([128, 128], bf16, name=f"vtp_{ci}_{kvh}_{b}",
                             tag="pv", bufs=2)
                nc.tensor.matmul(
                    tp[:],
                    lhsT=raw[:, b * 128 : (b + 1) * 128],
                    rhs=c["identb"][:],
                    is_transpose=True,
                )
                nc.vector.tensor_copy(vt[:, b * 128 : (b + 1) * 128], tp[:])
            g.vtok[(ci, kvh)] = vt
            for f in flush:
                f()
            yield
            continue

        is_q = ot >= 4
        # sumsq over head-dim partitions (RoPE-invariant -> use raw tile)
        sqh = []
        for sh in range(2):
            t = sb.tile([128, 512], bf16, name=f"sq_{ci}_{ot}_{sh}", tag="sq",
                        bufs=2)
            nc.scalar.activation(t[:], raw[:, sh * 512 : (sh + 1) * 512],
                                 mybir.ActivationFunctionType.Square)
            sqh.append(t)
        smt = ps.tile([128, 512], f32, name=f"ssq_{ci}_{ot}", tag="sm", bufs=1)
        if is_q:
            h = ot - 4
            # rows 0 / 32 of one PSUM bank hold the two 512-halves
            nc.tensor.matmul(smt[0:1, :], lhsT=c["onesb"][:], rhs=sqh[0][:])
            nc.tensor.matmul(smt[32:33, :], lhsT=c["onesb"][:], rhs=sqh[1][:])
            lnr = sb.tile([1, CHUNK], f32, name=f"lnq_{ci}_{ot}", tag="rowq",
                          bufs=2)
            nc.scalar.activation(
                lnr[:, 0:512], smt[0:1, :], mybir.ActivationFunctionType.Ln,
                bias=c["beps"][0:1, :], scale=float(1.0 / HD)
            )
            nc.scalar.activation(
                lnr[:, 512:1024], smt[32:33, :],
                mybir.ActivationFunctionType.Ln,
                bias=c["beps"][0:1, :], scale=float(1.0 / HD)
            )
            rows = sb.tile([1, CHUNK], bf16, name=f"rqr_{ci}_{ot}", tag="rowq",
                           bufs=2)
            nc.scalar.activation(
                rows[:], lnr[:], mybir.ActivationFunctionType.Exp, scale=-0.5
            )
            # replicate rows to all partitions via DMA broadcast
            bc = sb.tile([128, CHUNK], bf16, name=f"bcq_{ci}_{ot}", tag="bc",
                         bufs=3)
            nc.gpsimd.partition_broadcast(bc[:], rows[:])
        else:
            kvh = ot
            for b in range(NB):
                nc.tensor.matmul(
                    smt[:, b : b + 1],
                    lhsT=sqh[b // 4][:, (b % 4) * 128 : (b % 4 + 1) * 128],
                    rhs=c["onesb"][:],
                )
            # ln(mean+eps); rks = exp(-0.5*ln + ln(1/sqrt(HD))) emitted at the
            # attention boundary (same EXP table as the softmax).
            lnk = sb.tile([128, NB], f32, name=f"lnk_{ci}_{kvh}", tag="rk",
                          bufs=10)
            nc.scalar.activation(
                lnk[:], smt[:, 0:NB], mybir.ActivationFunctionType.Ln,
                bias=c["beps"][:, :], scale=float(1.0 / HD)
            )
            g.rkln[(ci, kvh)] = lnk

        # RoPE (fp32 math): fin <- (raw*cos + (R @ raw)*sin) [*1/rms for q]
        fin = sb.tile([128, CHUNK], bf16, name=f"fin_{ci}_{ot}", tag="qb",
                      bufs=14)
        for sh in range(2):
            rot = ps.tile([128, 512], f32, name=f"rot_{ci}_{ot}_{sh}",
                          tag="pv", bufs=2)
            nc.tensor.matmul(
                rot[:], lhsT=c["rmat"][:], rhs=raw[:, sh * 512 : (sh + 1) * 512]
            )
            nc.vector.tensor_tensor(
                rot[:], rot[:],
                g.c["sintab"][:, s0 + sh * 512 : s0 + (sh + 1) * 512],
                mybir.AluOpType.mult,
            )
            nc.vector.tensor_tensor(
                raw[:, sh * 512 : (sh + 1) * 512],
                raw[:, sh * 512 : (sh + 1) * 512],
                g.c["costab"][:, s0 + sh * 512 : s0 + (sh + 1) * 512],
                mybir.AluOpType.mult,
            )
            nc.vector.tensor_tensor(
                fin[:, sh * 512 : (sh + 1) * 512],
                raw[:, sh * 512 : (sh + 1) * 512],
                rot[:],
                mybir.AluOpType.add,
            )
        if is_q:
            def fin_q(fin=fin, bc=bc):
                nc.vector.tensor_tensor(
                    fin[:], fin[:], bc[:], mybir.AluOpType.mult
                )
            pending.append(fin_q)
        g.fin[(ci, ot)] = fin
        for f in flush:
            f()
        yield
    for f in pending:
        f()


# ---------------------------------------------------------------------------
# Phase 2: attention (block-causal within chunk)
# ---------------------------------------------------------------------------

def gen_attn(g, ci):
    """Yields once per (head, j-block) plus once per head tail."""
    nc, sb, ps = g.nc, g.sb, g.ps
    c = g.c
    hpending = []    # deferred at-normalize closures (bcast long done)
    den_pending = []  # deferred denominator chains for the previous head
    jpending = []    # deferred mask+sums+pv for the previous j-block
    # one PSUM bank of sum rows for the whole phase; head parity picks rows
    # (0,32) vs (64,96) so consecutive heads don't serialize.
    smt = ps.tile([128, 512], f32, name=f"sm_{ci}", tag="sm", bufs=1)
    # rks = exp(-0.5*ln(mean+eps) + ln(1/sqrt(HD))) = 1/(rms*sqrt(HD));
    # both kv heads upfront so no exp ever waits on them mid-phase
    rks_t = {}
    for kvh in range(KH):
        rks_t[kvh] = sb.tile([128, NB], f32, name=f"rks_{ci}_{kvh}", tag="rk",
                             bufs=10)
        nc.scalar.activation(
            rks_t[kvh][:], g.rkln[(ci, kvh)][:],
            mybir.ActivationFunctionType.Exp,
            bias=c["blniq"][:, :], scale=-0.5,
        )
    for kvh in range(KH):
        kf = g.fin[(ci, kvh)]
        vt = g.vtok[(ci, kvh)]
        rks = rks_t[kvh]
        for h4 in range(4):
            h = kvh * 4 + h4
            qf = g.fin[(ci, 4 + h)]
            pvA = ps.tile([128, 512], f32, name=f"pvA_{ci}_{h}", tag="pv",
                          bufs=2)
            pvB = ps.tile([128, 512], f32, name=f"pvB_{ci}_{h}", tag="pv",
                          bufs=2)
            ra = (h % 2) * 64
            rb = ra + 32
            at = sb.tile([128, CHUNK], SDT, name=f"attn_{ci}_{h}", tag="attn",
                         bufs=18)
            for j in range(NB):
                w = CHUNK - j * 128
                c0 = j * 128
                lenA = 512 - c0 if j < 4 else 0
                b0 = max(512, c0)
                sc = ps.tile([128, CHUNK], f32, name=f"sc_{ci}_{h}_{j}",
                             tag="sc", bufs=1)
                pt = sb.tile([128, CHUNK], bf16, name=f"pt_{ci}_{h}_{j}",
                             tag="pt", bufs=3)
                if lenA > 0:
                    nc.tensor.matmul(
                        sc[:, c0:512],
                        lhsT=kf[:, c0 : c0 + 128],
                        rhs=qf[:, c0:512],
                    )
                    nc.tensor.matmul(
                        sc[:, 512:CHUNK],
                        lhsT=kf[:, c0 : c0 + 128],
                        rhs=qf[:, 512:CHUNK],
                    )
                    # one exp over the contiguous [c0:1024) score range
                    nc.scalar.activation(
                        pt[:, 0:w], sc[:, c0:CHUNK],
                        mybir.ActivationFunctionType.Exp,
                        scale=rks[:, j : j + 1],
                    )
                else:
                    nc.tensor.matmul(
                        sc[:, 0:w],
                        lhsT=kf[:, c0 : c0 + 128],
                        rhs=qf[:, b0:CHUNK],
                    )
                    nc.scalar.activation(
                        pt[:, 0:w], sc[:, 0:w],
                        mybir.ActivationFunctionType.Exp,
                        scale=rks[:, j : j + 1],
                    )
                if j == 0:
                    # with this head's first exp already queued on ACT, emit
                    # the previous head's denominator chain (ACT) and the
                    # head-before's at-normalize (DVE) off the critical path
                    fl, dl = hpending[:], den_pending[:]
                    hpending.clear()
                    den_pending.clear()
                    for f in fl:
                        f()
                    for f in dl:
                        f()
                # previous j-block's mask+sums+pv: deferred one step so the
                # DVE mask op has had a full interleave round to complete
                fj = jpending[:]
                jpending.clear()
                for f in fj:
                    f()
                yield  # interleave point: ACT computes exp meanwhile

                def jwork(j=j, w=w, c0=c0, lenA=lenA, b0=b0, pt=pt,
                          pvA=pvA, pvB=pvB, at=at, ra=ra, rb=rb, vt=vt):
                    nc.vector.tensor_tensor(
                        pt[:, 0:128], pt[:, 0:128], c["mask"][:],
                        mybir.AluOpType.mult
                    )
                    if lenA > 0:
                        nc.tensor.matmul(
                            smt[ra : ra + 1, c0:512], lhsT=c["onesb"][:],
                            rhs=pt[:, 0:lenA],
                            start=(j == 0), stop=(j == 3),
                            tile_position=(0, ra),
                        )
                    nc.tensor.matmul(
                        smt[rb : rb + 1, b0 - 512 : 512], lhsT=c["onesb"][:],
                        rhs=pt[:, b0 - c0 : w],
                        start=(j == 0), stop=(j == NB - 1),
                        tile_position=(0, rb),
                    )
                    if lenA > 0:
                        nc.tensor.matmul(
                            pvA[:, c0:512],
                            lhsT=vt[:, c0 : c0 + 128],
                            rhs=pt[:, 0:lenA],
                            start=(j == 0), stop=(j == 3),
                        )
                        if j == 3:
                            # evacuate the finished A half right away to free
                            # the PSUM bank for the next head
                            nc.vector.tensor_copy(at[:, 0:512], pvA[:])
                    nc.tensor.matmul(
                        pvB[:, b0 - 512 : 512],
                        lhsT=vt[:, c0 : c0 + 128],
                        rhs=pt[:, b0 - c0 : w],
                        start=(j == 0), stop=(j == NB - 1),
                    )
                    if j == NB - 1:
                        nc.vector.tensor_copy(at[:, 512:1024], pvB[:])
                jpending.append(jwork)
            for f in jpending:
                f()
            jpending.clear()
            # denominators: 1/sum = exp(-ln(sum)), both halves in one op via
            # the packed sum rows (partitions ra and ra+32); emitted deferred
            # at the NEXT head's j=0 so these ACT ops never delay an exp.
            def den(h=h, ra=ra, rb=rb, at=at):
                lns = sb.tile([1, CHUNK], f32, name=f"lns_{ci}_{h}",
                              tag="rowq", bufs=2)
                nc.scalar.activation(lns[:, 0:512], smt[ra : ra + 1, :],
                                     mybir.ActivationFunctionType.Ln)
                nc.scalar.activation(lns[:, 512:1024], smt[rb : rb + 1, :],
                                     mybir.ActivationFunctionType.Ln)
                srows = sb.tile([1, CHUNK], bf16, name=f"srows_{ci}_{h}",
                                tag="srow", bufs=2)
                nc.scalar.activation(
                    srows[:], lns[:], mybir.ActivationFunctionType.Exp,
                    scale=-1.0
                )
                bcs = sb.tile([128, CHUNK], bf16, name=f"bcs_{ci}_{h}",
                              tag="bc", bufs=3)
                nc.gpsimd.partition_broadcast(bcs[:], srows[:])

                def fin_head(at=at, bcs=bcs):
                    nc.vector.tensor_tensor(
                        at[:], at[:], bcs[:], mybir.AluOpType.mult
                    )
                hpending.append(fin_head)
            den_pending.append(den)
            g.at[(ci, h)] = at
            yield
    for f in den_pending:
        f()
    for f in hpending:
        f()


# ---------------------------------------------------------------------------
# Phase 3: output projection (y DMA'd straight from PSUM)
# ---------------------------------------------------------------------------

def gen_wo(g, ci):
    """Yields 4x per dd tile (after every 4 accumulation matmuls)."""
    nc, sb, ps = g.nc, g.sb, g.ps
    s0 = ci * CHUNK
    wobs = {}

    def wob_load(dd):
        if dd not in wobs and dd < 32:
            t = sb.tile([128, QO], SDT, name=f"wo_{ci}_{dd}", tag="wo", bufs=4)
            nc.gpsimd.dma_start(t[:], g.woT[dd])
            wobs[dd] = t
        return wobs.get(dd)

    for dd in range(32):
        wob = wob_load(dd)
        wob_load(dd + 1)  # double-buffer the next weight block
        ysb = sb.tile([128, CHUNK], f32, name=f"ysb_{ci}_{dd}", tag="y", bufs=2)
        for sh in range(2):
            yps = ps.tile([128, 512], f32, name=f"y_{ci}_{dd}_{sh}",
                          tag="pacc", bufs=3)
            for h in range(QH):
                nc.tensor.matmul(
                    yps[:],
                    lhsT=wob[:, h * 128 : (h + 1) * 128],
                    rhs=g.at[(ci, h)][:, sh * 512 : (sh + 1) * 512],
                    start=(h == 0), stop=(h == QH - 1),
                )
                if h % 4 == 3:
                    yield
            nc.vector.tensor_copy(
                ysb[:, sh * 512 : (sh + 1) * 512], yps[:]
            )
        nc.sync.dma_start(
            g.yT[dd * 128 : (dd + 1) * 128, s0 : s0 + CHUNK], ysb[:]
        )


def _interleave(main, aux, n_main, n_aux):
    """Drive main to exhaustion, advancing aux proportionally (n_aux aux
    steps spread over n_main main steps)."""
    done_aux = 0
    i = 0
    while True:
        if next(main, _DONE) is _DONE:
            break
        i += 1
        tgt = min(n_aux, i * n_aux // max(1, n_main))
        if tgt > done_aux:
            if _drive(aux, tgt - done_aux):
                done_aux = tgt
            else:
                done_aux = n_aux


def _build_program():
    nc = bacc.Bacc(
        "TRN2",
        target_bir_lowering=False,
        debug=False,
        enable_asserts=False,
        num_devices=NCORES,
    )
    xT = nc.dram_tensor("xT", [D, TOK], SDT, kind="ExternalInput").ap()
    # [ot, p, dk, c]: per (ot, pass) slice is one contiguous-per-partition DMA
    wqkvT = nc.dram_tensor(
        "wqkvT", [NOT, 128, D // 128, 128], SDT, kind="ExternalInput"
    ).ap()
    # per-dd-block tiled layout: woT[dd, p, h*128+c] = wo[dd*128+c, hg*QO + h*128+p]
    woT = nc.dram_tensor("woT", [D // 128, 128, QO], SDT, kind="ExternalInput").ap()
    costab = nc.dram_tensor("costab", [HD, TOK], bf16, kind="ExternalInput").ap()
    sintab = nc.dram_tensor("sintab", [HD, TOK], bf16, kind="ExternalInput").ap()
    rmat = nc.dram_tensor("rmat", [128, 128], bf16, kind="ExternalInput").ap()
    identb = nc.dram_tensor("identb", [128, 128], bf16, kind="ExternalInput").ap()
    mask = nc.dram_tensor("mask", [128, 128], bf16, kind="ExternalInput").ap()
    onesb = nc.dram_tensor("onesb", [128, 1], bf16, kind="ExternalInput").ap()
    beps = nc.dram_tensor("beps", [128, 1], f32, kind="ExternalInput").ap()
    blniq = nc.dram_tensor("blniq", [128, 1], f32, kind="ExternalInput").ap()
    yT = nc.dram_tensor("yT", [D, TOK], f32, kind="ExternalOutput").ap()

    with tile.TileContext(nc) as tc, ExitStack() as ctx:
        ctx.enter_context(nc.allow_low_precision(reason="bf16 attention operands"))
        sb = ctx.enter_context(tc.tile_pool(name="sb", bufs=1))
        ps = ctx.enter_context(tc.tile_pool(name="ps", bufs=1, space="PSUM"))
        cp = ctx.enter_context(tc.tile_pool(name="cp", bufs=1))

        consts = {}
        for nm, ap_, shape, dt_ in (
            ("costab", costab, [HD, TOK], bf16),
            ("sintab", sintab, [HD, TOK], bf16),
            ("rmat", rmat, [128, 128], bf16),
            ("identb", identb, [128, 128], bf16),
            ("mask", mask, [128, 128], bf16),
            ("onesb", onesb, [128, 1], bf16),
            ("beps", beps, [128, 1], f32),
            ("blniq", blniq, [128, 1], f32),
        ):
            t = cp.tile(shape, dt_, name=f"c_{nm}")
            nc.scalar.dma_start(t[:], ap_[:])
            consts[nm] = t

        g = Ctx(nc, sb, ps, consts, (xT, wqkvT, woT, yT))

        # P0 with inline post
        _drain(gen_proj(g, 0, inline_post=True))
        # A0 (x) P1raw: ~72 attn steps over 194 proj yield units
        gp1 = gen_proj(g, 1, inline_post=False)
        _interleave(gen_attn(g, 0), gp1, n_main=72, n_aux=194)
        _drain(gp1)
        # post1 (x) first dds of W0: 12 post steps over 44 wo units (11 dd)
        gw0 = gen_wo(g, 0)
        _drive(gw0, 4)  # first dd fully in flight before post1's chains
        _interleave(gen_post(g, 1), gw0, n_main=12, n_aux=52)
        # A1 (x) rest of W0 (~72 wo units over 72 attn steps)
        _interleave(gen_attn(g, 1), gw0, n_main=72, n_aux=72)
        _drain(gw0)
        # W1
        _drain(gen_wo(g, 1))

    nc.compile()
    return nc


def _host_inputs(x, wq, wk, wv, wo):
    xf = np.ascontiguousarray(x.reshape(S, D).T.astype(SNP))  # [D, S]
    half = HD // 2
    inv_freq = (1.0 / (THETA ** (np.arange(0, half, dtype=np.float32) / half))).astype(
        np.float32
    )
    ang = np.arange(S, dtype=np.float32)[:, None] * inv_freq[None, :]
    cos = np.cos(ang).astype(np.float32)
    sin = np.sin(ang).astype(np.float32)
    costab = np.empty((HD, S), np.float32)
    sintab = np.empty((HD, S), np.float32)
    costab[0::2, :] = cos.T
    costab[1::2, :] = cos.T
    sintab[0::2, :] = sin.T
    sintab[1::2, :] = sin.T
    costab = costab.astype(ml_dtypes.bfloat16)
    sintab = sintab.astype(ml_dtypes.bfloat16)

    rmat = np.zeros((128, 128), ml_dtypes.bfloat16)
    for i in range(64):
        rmat[2 * i + 1, 2 * i] = -1.0
        rmat[2 * i, 2 * i + 1] = 1.0
    identb = np.eye(128, dtype=ml_dtypes.bfloat16)
    mask = np.triu(np.ones((128, 128), np.float32)).astype(ml_dtypes.bfloat16)
    onesb = np.ones((128, 1), ml_dtypes.bfloat16)
    beps = np.full((128, 1), EPS, np.float32)
    blniq = np.full((128, 1), np.log(ISQ_HD), np.float32)

    xT_cg = [np.ascontiguousarray(xf[:, cg * TOK : (cg + 1) * TOK]) for cg in range(CG)]
    cos_cg = [np.ascontiguousarray(costab[:, cg * TOK : (cg + 1) * TOK]) for cg in range(CG)]
    sin_cg = [np.ascontiguousarray(sintab[:, cg * TOK : (cg + 1) * TOK]) for cg in range(CG)]
    wqkvT_hg = []
    woT_hg = []
    for hg in range(HG):
        wq_c = wq[hg * QO : (hg + 1) * QO]
        wk_c = wk[hg * KO : (hg + 1) * KO]
        wv_c = wv[hg * KO : (hg + 1) * KO]
        # o-tile order on device: [k0 k1 v0 v1 q0..q7]
        wcat = np.concatenate([wk_c, wv_c, wq_c], 0)  # [QKVO, D]
        # wqkvT[ot, p, dk, c] = wcat[ot*128 + c?, ...]: lhsT tile for (ot, dk)
        # is w[dk*128+p(part), ot*128+c(free)] = wcat[ot*128+c, dk*128+p].
        wr = wcat.T.reshape(D // 128, 128, NOT, 128)  # [dk, p, ot, c]
        wr = wr.transpose(2, 1, 0, 3)  # [ot, p, dk, c]
        wqkvT_hg.append(np.ascontiguousarray(wr.astype(SNP)))
        wo_c = wo[:, hg * QO : (hg + 1) * QO]  # [D, QO]
        woH = wo_c.reshape(D // 128, 128, QH, 128).transpose(0, 3, 2, 1)  # [dd, p, hb, c]
        woT_hg.append(np.ascontiguousarray(woH.reshape(D // 128, 128, QO).astype(SNP)))

    in_maps = []
    for c in range(NCORES):
        cg, hg = c // HG, c % HG
        in_maps.append(
            {
                "xT": xT_cg[cg],
                "wqkvT": wqkvT_hg[hg],
                "woT": woT_hg[hg],
                "costab": cos_cg[cg],
                "sintab": sin_cg[cg],
                "rmat": rmat,
                "identb": identb,
                "mask": mask,
                "onesb": onesb,
                "beps": beps,
                "blniq": blniq,
            }
        )
    return in_maps


def _assemble(results):
    y = np.empty((S, D), np.float32)
    for cg in range(CG):
        acc = results[cg * HG]["yT"].astype(np.float32)
        for hg in range(1, HG):
            acc = acc + results[cg * HG + hg]["yT"]
        y[cg * TOK : (cg + 1) * TOK, :] = acc.T
    return y.reshape(1, S, D)


def kernel(x, wq, wk, wv, wo, **_kw):
    x = np.asarray(x, np.float32)
    wq = np.asarray(wq, np.float32)
    wk = np.asarray(wk, np.float32)
    wv = np.asarray(wv, np.float32)
    wo = np.asarray(wo, np.float32)

    if "nc" not in _CACHE:
        _CACHE["nc"] = _build_program()
    nc = _CACHE["nc"]
    in_maps = _host_inputs(x, wq, wk, wv, wo)
    res = run_bass_kernel_spmd(nc, in_maps, core_ids=list(range(NCORES)))
    _CACHE["last_result"] = res
    return _assemble(res.results)


def run_traced(x, wq, wk, wv, wo, tmpdir=None):
    """Like kernel() but with NTFF tracing; returns (out, BassKernelResults)."""
    if "nc" not in _CACHE:
        _CACHE["nc"] = _build_program()
    nc = _CACHE["nc"]
    in_maps = _host_inputs(
        np.asarray(x, np.float32), np.asarray(wq, np.float32),
        np.asarray(wk, np.float32), np.asarray(wv, np.float32),
        np.asarray(wo, np.float32),
    )
    res = run_bass_kernel_spmd(
        nc, in_maps, core_ids=list(range(NCORES)), trace=True, tmpdir=tmpdir
    )
    return _assemble(res.results), res
